# revision 12
# baseline (speedup 1.0000x reference)
"""Trainium2 Bass kernel for the MACE charge-equilibration module (nn_MACE_44435731645168).

Mathematical restructuring (exact, validated against the jax reference):
  * batch = repeat(arange(64), 64) and every edge connects atoms of the same
    graph, so the dense TxT distance matrix is EPS everywhere except inside the
    64 diagonal 64x64 blocks.  Off-block entries of `factor` and `F_cut` are the
    analytic constants c_off = erf(1/sqrt(2))/EPS and c1 = tanh(1)^3, and the
    off-block part of each dense matvec collapses to c_off*(S_tot - S_g) with
    S_g the per-graph charge sum.  All remaining work is per-graph 64x64 blocks.
  * The extended (N+1)x(N+1) solve [[L,1],[1^T,0]] [q;lam] = [-kappa;0] with L
    lower-triangular reduces to two triangular solves: u = L^-1 kappa,
    v = L^-1 1, q = -u + (sum(u)/sum(v)) v.  Both are done for all 64 graphs at
    once on-device: graphs x {u,v} occupy the 128 SBUF partitions and each
    forward-substitution step is a single fused multiply-reduce instruction.

Device work: erf/tanh/exp transcendentals for all pair blocks, the 64-step
batched triangular solve, the blocked matvecs/reductions, and the PE matmuls
that shuffle charges between layouts.  Host work: dtype/layout prep, scattering
the edge list into dense 64x64 blocks (last-write-wins, matching XLA scatter),
and expanding the tiny 4x4 type-pair parameter tables.

Each of the 8 cores runs an identical program; per-core inputs differ only in
which 8 graphs (512 atoms) the core computes outputs for.  The (tiny) solve is
replicated on every core, which removes all cross-core communication.
"""

import math

import numpy as np

try:
    from scipy.special import erf as _erf
except Exception:  # pragma: no cover
    _erf = np.vectorize(math.erf, otypes=[np.float64])

G = 64
N = 64
T = G * N
P = 128
NCORES = 8
GPC = G // NCORES        # graphs per core
APC = T // NCORES        # atoms per core
S_SUB = APC // P         # free-dim sub-blocks in the per-atom layout (4)
EPS = np.float32(0.5)
R_MAX = np.float32(6.0)
R_IN = np.float32(1.0)
SQRT_PI = np.float32(np.sqrt(np.pi))
C1 = np.float32(np.tanh(1.0) ** 3)
C_OFF = np.float32(_erf(1.0 / np.sqrt(2.0)) / 0.5)

F32 = np.float32

# bpack free-dim slices ([128, 256] each)
BW = S_SUB * N  # 256
_BP_NAMES = ["xargB", "invRB", "tharg", "earg", "cd", "diag01"]
BP = BW * len(_BP_NAMES)
# spack free-dim layout: shift_sel [0:64], halves [64:66], dterm [66:70], ase [70:74]
SP = 74

# packed lower-triangle A layout: row j occupies [tri(j), tri(j)+j+1) with
# slot 0 carrying b_tilde and slots 1..j carrying -L[j,k]/L[j,j], k<j
TRI_OFF = np.cumsum([0] + [j + 1 for j in range(N)])  # [65]; TRI_OFF[64]=2080
TRI = int(TRI_OFF[N])
_RIDX = np.repeat(np.arange(N), np.arange(1, N + 1))          # [TRI]
_CIDX = np.concatenate([np.arange(j + 1) for j in range(N)])  # [TRI]

_compiled = None


def _a_chunks(nch):
    """Split packed rows 0..63 into ~nch chunks balanced by element count."""
    bounds = [0]
    target = TRI / nch
    for j in range(1, N):
        if TRI_OFF[j] >= target * len(bounds) and bounds[-1] != j:
            bounds.append(j)
    bounds.append(N)
    return [(bounds[i], bounds[i + 1]) for i in range(len(bounds) - 1)]


def _build():
    """Build + compile the Bass program once. Returns (nc, meta)."""
    import concourse.bacc as bacc
    import concourse.tile as tile
    import concourse.mybir as mybir
    from concourse import bass

    dt = mybir.dt.float32
    Act = mybir.ActivationFunctionType
    Op = mybir.AluOpType
    Ax = mybir.AxisListType

    nc = bacc.Bacc("TRN2", target_bir_lowering=False, debug=False,
                   num_devices=NCORES)

    xarg_a = nc.dram_tensor("xarg_a", [P, TRI], dt, kind="ExternalInput").ap()
    w_a = nc.dram_tensor("w_a", [P, TRI], dt, kind="ExternalInput").ap()
    bpack = nc.dram_tensor("bpack", [P, BP], dt, kind="ExternalInput").ap()
    spack = nc.dram_tensor("spack", [P, SP], dt, kind="ExternalInput").ap()
    selq = nc.dram_tensor("selq", [G, S_SUB * P], dt, kind="ExternalInput").ap()

    q_out = nc.dram_tensor("q_out", [G, N], dt, kind="ExternalOutput").ap()
    pot_out = nc.dram_tensor("pot_out", [P, S_SUB], dt, kind="ExternalOutput").ap()
    ne_out = nc.dram_tensor("ne_out", [P, S_SUB], dt, kind="ExternalOutput").ap()
    sums_out = nc.dram_tensor("sums_out", [2, 8], dt, kind="ExternalOutput").ap()

    chunks = _a_chunks(6)   # packed A-stage row-group chunks

    def bslice(tile_ap, name):
        i = _BP_NAMES.index(name)
        return tile_ap[:, i * BW:(i + 1) * BW]

    with tile.TileContext(nc) as tc:
        with tc.tile_pool(name="main", bufs=1) as pool, \
             tc.tile_pool(name="chunks", bufs=3) as cpool, \
             tc.tile_pool(name="psum", bufs=2, space="PSUM") as pp:

            # ---------------- input DMAs ----------------
            sp = pool.tile([P, SP], dt)
            nc.sync.dma_start(out=sp[:], in_=spack[:])
            bp = pool.tile([P, BP], dt)
            nc.sync.dma_start(out=bp[:], in_=bpack[:])
            sel = pool.tile([G, S_SUB * P], dt)
            nc.sync.dma_start(out=sel[:], in_=selq[:])

            # ---------------- A stage: packed Lneg chunks ----------------
            # Packed rows: row j at [tri(j), tri(j)+j+1); slot 0 carries
            # b_tilde (via w = b_tilde/erf(1)), slots 1..j carry
            # -L[j,k]/L[j,j] for k<j.
            lneg = {}
            for (r0, r1) in chunks:
                o0, o1 = int(TRI_OFF[r0]), int(TRI_OFF[r1])
                cw = o1 - o0
                xa = cpool.tile([P, cw], dt, tag="xa")
                nc.sync.dma_start(out=xa[:], in_=xarg_a[:, o0:o1])
                wa = cpool.tile([P, cw], dt, tag="wa")
                nc.sync.dma_start(out=wa[:], in_=w_a[:, o0:o1])
                ea = cpool.tile([P, cw], dt, tag="ea")
                nc.scalar.activation(out=ea[:], in_=xa[:], func=Act.Erf)
                ln = pool.tile([P, cw], dt, tag=f"lneg{r0}")
                nc.vector.tensor_tensor(out=ln[:], in0=ea[:], in1=wa[:],
                                        op=Op.mult)
                for j in range(r0, r1):
                    lneg[j] = (ln, int(TRI_OFF[j]) - o0)

            # ---------------- batched forward substitution ----------------
            # partitions = graph g (rows 0-63, RHS u) and g+64 (RHS v)
            # y layout: col 0 = constant 1, cols 1..64 = solution, col 65 = sum
            y = pool.tile([P, N + 2], dt)
            scratch = pool.tile([P, N], dt)
            nc.vector.memset(y[:, 0:1], 1.0)
            for j in range(N):
                ln, off = lneg[j]
                nc.vector.scalar_tensor_tensor(
                    out=scratch[:, 0:j + 1],
                    in0=ln[:, off:off + j + 1],
                    scalar=1.0,
                    in1=y[:, 0:j + 1],
                    op0=Op.bypass,
                    op1=Op.mult,
                    accum_out=y[:, j + 1:j + 2],
                )
            # row sums -> y[:, 65]  (Sum u per graph / Sum v per graph)
            nc.vector.tensor_reduce(out=y[:, N + 1:N + 2], in_=y[:, 1:N + 1],
                                    axis=Ax.X, op=Op.add)

            # ---------------- q = -u + (Su/Sv) v ----------------
            vsh = pp.tile([G, N + 1], dt)
            nc.tensor.matmul(vsh[:], lhsT=sp[:, 0:64], rhs=y[:, 1:N + 2],
                             start=True, stop=True)
            rv = pool.tile([G, 1], dt)
            nc.vector.reciprocal(out=rv[:], in_=vsh[:, N:N + 1])
            r = pool.tile([G, 1], dt)
            nc.vector.tensor_tensor(out=r[:], in0=y[0:G, N + 1:N + 2], in1=rv[:],
                                    op=Op.mult)
            # qrhs: cols 0..63 = charges per graph, col 64 = S_tot - S_g
            qrhs = pool.tile([G, N + 1], dt)
            qa = qrhs[:, 0:N]
            nc.vector.scalar_tensor_tensor(out=qa, in0=vsh[:, 0:N],
                                           scalar=r[:], in1=y[0:G, 1:N + 1],
                                           op0=Op.mult, op1=Op.subtract)
            nc.sync.dma_start(out=q_out[:], in_=qa)

            # S_tot broadcast, then offw = S_tot - S_g into qrhs col 64
            sga = pool.tile([G, 1], dt)
            nc.vector.tensor_reduce(out=sga[:], in_=qa, axis=Ax.X, op=Op.add)
            ones64 = pool.tile([G, 1], dt)
            nc.vector.memset(ones64[:], 1.0)
            stp = pp.tile([1, 1], dt)
            nc.tensor.matmul(stp[:], lhsT=sga[:], rhs=ones64[:],
                             start=True, stop=True)
            sts = pool.tile([1, 1], dt)
            nc.vector.tensor_copy(out=sts[:], in_=stp[:])
            stb = pool.tile([G, 1], dt)
            nc.gpsimd.partition_broadcast(stb[:], sts[:])
            nc.vector.tensor_scalar(out=qrhs[:, N:N + 1], in0=sga[:],
                                    scalar1=stb[:], scalar2=-1.0,
                                    op0=Op.subtract, op1=Op.mult)

            # ---------------- charges + offw in per-atom layout ----------
            W1 = N + 1
            qbp = pp.tile([P, S_SUB * W1], dt)
            for s in range(S_SUB):
                nc.tensor.matmul(qbp[:, s * W1:(s + 1) * W1],
                                 lhsT=sel[:, s * P:(s + 1) * P], rhs=qrhs[:],
                                 start=True, stop=True)
            qbs = pool.tile([P, S_SUB * W1], dt)
            nc.vector.tensor_copy(out=qbs[:], in_=qbp[:])
            qb3 = qbs[:].rearrange("p (s w) -> p s w", w=W1)
            qB = qb3[:, :, 0:N]                   # [128, 4, 64] charges
            offw = qb3[:, :, N:N + 1]             # [128, 4, 1]  S_tot - S_g

            # ---------------- B stage (per-atom rows, this core's atoms) ----
            eb = pool.tile([P, BW], dt)
            nc.scalar.activation(out=eb[:], in_=bslice(bp, "xargB"), func=Act.Erf)
            fac = pool.tile([P, BW], dt)
            nc.gpsimd.tensor_tensor(out=fac[:], in0=eb[:],
                                    in1=bslice(bp, "invRB"), op=Op.mult)
            th = pool.tile([P, BW], dt)
            nc.scalar.activation(out=th[:], in_=bslice(bp, "tharg"), func=Act.Tanh)
            th2 = pool.tile([P, BW], dt)
            nc.gpsimd.tensor_tensor(out=th2[:], in0=th[:], in1=th[:], op=Op.mult)
            fcut = pool.tile([P, BW], dt)
            nc.gpsimd.tensor_tensor(out=fcut[:], in0=th2[:], in1=th[:], op=Op.mult)
            ex = pool.tile([P, BW], dt)
            nc.scalar.activation(out=ex[:], in_=bslice(bp, "earg"), func=Act.Exp)
            t2b = pool.tile([P, BW], dt)
            nc.gpsimd.tensor_tensor(out=t2b[:], in0=ex[:], in1=bslice(bp, "cd"),
                                    op=Op.subtract)
            t2b2 = pool.tile([P, BW], dt)
            nc.gpsimd.tensor_tensor(out=t2b2[:], in0=t2b[:], in1=fcut[:],
                                    op=Op.mult)

            sums_in = pool.tile([P, 8], dt)
            e2brow = sums_in[:, 4:8]
            nc.vector.tensor_reduce(
                out=e2brow, in_=t2b2[:].rearrange("p (s l) -> p s l", l=N),
                axis=Ax.X, op=Op.add)

            # matvecs: V_pre = sum(fac*q), P = sum(fac*q*Fc) per atom row
            tmpv = pool.tile([P, BW], dt)
            tv3 = tmpv[:].rearrange("p (s l) -> p s l", l=N)
            nc.vector.tensor_tensor(out=tv3, in0=fac[:].rearrange(
                "p (s l) -> p s l", l=N), in1=qB, op=Op.mult)
            vpre = pool.tile([P, S_SUB], dt)
            nc.vector.tensor_reduce(out=vpre[:], in_=tv3, axis=Ax.X, op=Op.add)
            tmpp = pool.tile([P, BW], dt)
            nc.vector.tensor_tensor(out=tmpp[:], in0=tmpv[:], in1=fcut[:],
                                    op=Op.mult)
            pb = pool.tile([P, S_SUB], dt)
            nc.vector.tensor_reduce(
                out=pb[:], in_=tmpp[:].rearrange("p (s l) -> p s l", l=N),
                axis=Ax.X, op=Op.add)

            # own-charge per atom via diagonal mask
            qd = pool.tile([P, BW], dt)
            nc.gpsimd.tensor_tensor(out=qd[:].rearrange("p (s l) -> p s l", l=N),
                                    in0=qB, in1=bslice(bp, "diag01").rearrange(
                                        "p (s l) -> p s l", l=N), op=Op.mult)
            qown = pool.tile([P, S_SUB], dt)
            nc.vector.tensor_reduce(
                out=qown[:], in_=qd[:].rearrange("p (s l) -> p s l", l=N),
                axis=Ax.X, op=Op.add)

            # atomic potentials: P + c1*c_off*(S_tot - S_g)
            pot = pool.tile([P, S_SUB], dt)
            nc.vector.scalar_tensor_tensor(
                out=pot[:].unsqueeze(-1), in0=offw, scalar=float(C1 * C_OFF),
                in1=pb[:].unsqueeze(-1), op0=Op.mult, op1=Op.add)
            nc.sync.dma_start(out=pot_out[:], in_=pot[:])

            # V, atomic electrostatic energy
            v1 = pool.tile([P, S_SUB], dt)
            nc.vector.scalar_tensor_tensor(
                out=v1[:].unsqueeze(-1), in0=offw, scalar=float(C_OFF),
                in1=vpre[:].unsqueeze(-1), op0=Op.mult, op1=Op.add)
            dq = pool.tile([P, S_SUB], dt)
            nc.gpsimd.tensor_tensor(out=dq[:], in0=qown[:], in1=sp[:, 66:70],
                                    op=Op.mult)
            vfin = pool.tile([P, S_SUB], dt)
            nc.vector.tensor_tensor(out=vfin[:], in0=v1[:], in1=dq[:], op=Op.add)
            aeel = sums_in[:, 0:4]
            nc.vector.tensor_tensor(out=aeel, in0=qown[:], in1=vfin[:], op=Op.mult)

            ne = pool.tile([P, S_SUB], dt)
            nc.vector.scalar_tensor_tensor(out=ne[:], in0=e2brow, scalar=0.5,
                                           in1=aeel, op0=Op.mult, op1=Op.add)
            ne2 = pool.tile([P, S_SUB], dt)
            nc.vector.tensor_tensor(out=ne2[:], in0=ne[:], in1=sp[:, 70:74],
                                    op=Op.add)
            nc.sync.dma_start(out=ne_out[:], in_=ne2[:])

            # per-graph sums of aeel / e2brow  (halves selector matmul)
            smp = pp.tile([2, 8], dt)
            nc.tensor.matmul(smp[:], lhsT=sp[:, 64:66], rhs=sums_in[:],
                             start=True, stop=True)
            sms = pool.tile([2, 8], dt)
            nc.vector.tensor_copy(out=sms[:], in_=smp[:])
            nc.sync.dma_start(out=sums_out[:], in_=sms[:])

    nc.compile()
    return nc


def _get_compiled():
    global _compiled
    if _compiled is None:
        _compiled = _build()
    return _compiled


def _f_cut_np(R):
    c1 = np.float32(np.tanh(1.0) ** 3)
    smooth = np.tanh((1.0 - (R - R_IN) / (R_MAX - R_IN)).astype(F32)).astype(F32) ** 3
    out = np.where((R > 0) & (R < R_IN), c1,
                   np.where((R >= R_IN) & (R <= R_MAX), smooth, F32(0.0)))
    return out.astype(F32)


def _fallback_numpy(batch, node_attrs, edge_index, edge_length, kappa,
                    node_feats, ref_eta, ref_log_sigma, ref_A, ref_B, ref_C,
                    ref_D, ref_mu, short_energy, atomic_short_energy):
    """Faithful numpy port of the reference for non-conforming inputs."""
    Tn = batch.shape[0]
    Gn = int(batch.max()) + 1 if Tn else 0
    Gn = max(Gn, short_energy.shape[0])
    Nn = Tn // Gn
    types = np.argmax(node_attrs, axis=1)
    ref_sigma = np.exp(ref_log_sigma).astype(F32)
    sigma = ref_sigma[types]
    eta = ref_eta[types]
    ref_gamma = np.sqrt(ref_sigma[:, None] ** 2 + ref_sigma[None, :] ** 2).astype(F32)
    same = batch[:, None] == batch[None, :]
    gamma = np.where(same, ref_gamma[types[:, None], types[None, :]], EPS).astype(F32)
    R = np.full((Tn, Tn), EPS, dtype=F32)
    R[edge_index[0], edge_index[1]] = edge_length[:, 0]
    Fc = _f_cut_np(R)
    factor = (_erf((R / (np.sqrt(F32(2.0)) * gamma)).astype(F32)).astype(F32) / R).astype(F32)
    A_ij = np.diag((eta + 1.0 / (sigma * SQRT_PI)).astype(F32)) + factor
    A_tril = np.tril(A_ij).astype(F32)
    A4 = A_tril.reshape(Gn, Nn, Gn, Nn)
    gi = np.arange(Gn)
    A_blocks = A4[gi, :, gi, :]
    A_ext = np.zeros((Gn, Nn + 1, Nn + 1), dtype=F32)
    A_ext[:, :Nn, :Nn] = A_blocks
    A_ext[:, Nn, :Nn] = 1.0
    A_ext[:, :Nn, Nn] = 1.0
    kappa_ext = np.concatenate([kappa.reshape(Gn, Nn),
                                np.zeros((Gn, 1), F32)], axis=1)
    sol = np.linalg.solve(A_ext.astype(np.float64),
                          -kappa_ext[..., None].astype(np.float64))[..., 0]
    charges = sol[:, :Nn].reshape(Tn).astype(F32)
    atomic_potentials = ((factor * Fc) @ charges).astype(F32)
    E_factor = np.diag((0.5 / (sigma * SQRT_PI)).astype(F32)) + factor
    V = (E_factor @ charges).astype(F32)
    atomic_E_el = (charges * V).astype(F32)
    E_el = np.zeros((Gn, 1), F32)
    np.add.at(E_el[:, 0], batch, atomic_E_el)
    pair = lambda ref: np.where(same, ref[types[:, None], types[None, :]], F32(0.0)).astype(F32)
    Am, Bm, Cm, Dm, mum = (pair(x) for x in (ref_A, ref_B, ref_C, ref_D, ref_mu))
    E2b_ij = ((Am * np.exp((Bm * (mum - R)).astype(F32)).astype(F32)
               - Cm / R ** 6 - Dm / R ** 8).astype(F32) * Fc)
    E2b_ij *= (1.0 - np.eye(Tn, dtype=F32))
    atomic_E2b = (E2b_ij.sum(axis=-1) * 0.5)[:, None].astype(F32)
    E_2b = np.zeros((Gn, 1), F32)
    np.add.at(E_2b[:, 0], batch, atomic_E2b[:, 0])
    E_tot = (E_el + E_2b + short_energy.astype(F32)).astype(F32)
    node_energy = (atomic_E_el[:, None] + atomic_E2b
                   + atomic_short_energy.astype(F32)).astype(F32)
    node_feats_out = np.concatenate(
        [node_feats.astype(F32), charges[:, None], atomic_potentials[:, None]],
        axis=1).astype(F32)
    return E_tot, node_energy, charges, node_feats_out


def _host_prep(batch, node_attrs, edge_index, edge_length, kappa, node_feats,
               ref_eta, ref_log_sigma, ref_A, ref_B, ref_C, ref_D, ref_mu,
               short_energy, atomic_short_energy):
    """Numpy-side layout/table prep. Returns per-core in_maps."""
    types = np.argmax(node_attrs, axis=1).astype(np.int64)            # [T]
    ref_sigma = np.exp(ref_log_sigma.astype(F32)).astype(F32)
    sigma = ref_sigma[types]
    eta = ref_eta.astype(F32)[types]
    ref_gamma = np.sqrt(ref_sigma[:, None] ** 2
                        + ref_sigma[None, :] ** 2).astype(F32)        # [4,4]

    # dense per-graph blocks of R (last-write-wins scatter, matches XLA)
    e0 = edge_index[0].astype(np.int64)
    e1 = edge_index[1].astype(np.int64)
    Rb = np.full((G, N, N), EPS, dtype=F32)
    Rb[e0 // N, e0 % N, e1 % N] = edge_length[:, 0].astype(F32)

    tb = types.reshape(G, N)                                          # [G,N]
    gam = ref_gamma[tb[:, :, None], tb[:, None, :]]                   # [G,N,N]
    inv_sqrt2g = (F32(1.0) / (np.sqrt(F32(2.0)) * gam)).astype(F32)
    xargA = (Rb * inv_sqrt2g).astype(F32)                             # [G,N,N]
    invR = (F32(1.0) / Rb).astype(F32)

    # diagonal of L and the row pre-scaling
    idx = np.arange(N)
    fdiag = (_erf(xargA[:, idx, idx]).astype(F32) * invR[:, idx, idx]).astype(F32)
    dvec = (eta + F32(1.0) / (sigma * SQRT_PI)).astype(F32).reshape(G, N)
    invd = (F32(1.0) / (dvec + fdiag)).astype(F32)                    # [G,N]
    wA = (-invd[:, :, None] * invR).astype(F32)                       # [G,N,N]

    # packed lower-triangle rows; slot 0 of row j carries b_tilde via
    # erf(1.0) * (b_tilde/erf(1.0)) = b_tilde
    erf1 = F32(_erf(np.float32(1.0)))
    bu = (invd * kappa.astype(F32).reshape(G, N) / erf1).astype(F32)
    bv = (invd / erf1).astype(F32)
    cid0 = np.maximum(_CIDX - 1, 0)
    half_x = np.where(_CIDX == 0, F32(1.0), xargA[:, _RIDX, cid0])    # [G,TRI]
    wgat = wA[:, _RIDX, cid0]
    w_u = np.where(_CIDX == 0, bu[:, _RIDX], wgat).astype(F32)
    w_v = np.where(_CIDX == 0, bv[:, _RIDX], wgat).astype(F32)
    xarg_a = np.concatenate([half_x, half_x], axis=0).astype(F32)     # [128,TRI]
    w_a = np.concatenate([w_u, w_v], axis=0)                          # [128,TRI]

    # ---- per-atom (B) layout, full problem then sliced per core ----
    gi_of = np.arange(T) // N
    li_of = np.arange(T) % N
    R_row = Rb[gi_of, li_of, :]                                       # [T,64]
    xargB = xargA[gi_of, li_of, :]
    invRB = invR[gi_of, li_of, :]
    tharg = (F32(1.2) - F32(0.2) * np.clip(R_row, R_IN, R_MAX)).astype(F32)

    t_i = types[:, None]                                              # [T,1]
    t_j = tb[gi_of]                                                   # [T,64]
    lnA = np.log(ref_A.astype(F32)).astype(F32)
    beta = ref_B.astype(F32)[t_i, t_j]
    delta = (beta * ref_mu.astype(F32)[t_i, t_j] + lnA[t_i, t_j]).astype(F32)
    earg = (delta - beta * R_row).astype(F32)
    iR2 = (invRB * invRB).astype(F32)
    iR6 = (iR2 * iR2 * iR2).astype(F32)
    cd = (ref_C.astype(F32)[t_i, t_j] * iR6
          + ref_D.astype(F32)[t_i, t_j] * iR6 * iR2).astype(F32)
    diag01 = (t_j * 0).astype(F32)
    diag01[np.arange(T), li_of] = 1.0
    earg[np.arange(T), li_of] = -100.0
    cd[np.arange(T), li_of] = 0.0

    dterm = (F32(0.5) / (sigma * SQRT_PI)).astype(F32)                # [T]
    ase = atomic_short_energy.astype(F32)[:, 0]                       # [T]

    # constant small tensors
    shift_sel = np.zeros((P, G), F32)
    shift_sel[np.arange(G) + G, np.arange(G)] = 1.0                   # k==m+64
    halves = np.zeros((P, 2), F32)
    halves[:G, 0] = 1.0
    halves[G:, 1] = 1.0

    def to_b_layout(arr_tc):  # [APC, 64] -> [128, 256], atom = s*128 + p
        return np.ascontiguousarray(
            arr_tc.reshape(S_SUB, P, N).transpose(1, 0, 2).reshape(P, BW))

    def to_b_small(vec):      # [APC] -> [128, 4]
        return np.ascontiguousarray(vec.reshape(S_SUB, P).T)

    in_maps = []
    for c in range(NCORES):
        rows = slice(c * APC, (c + 1) * APC)
        bpack = np.concatenate(
            [to_b_layout(a[rows]) for a in (xargB, invRB, tharg, earg, cd, diag01)],
            axis=1).astype(F32)
        spack = np.zeros((P, SP), F32)
        spack[:, 0:64] = shift_sel
        spack[:, 64:66] = halves
        spack[:, 66:70] = to_b_small(dterm[rows])
        spack[:, 70:74] = to_b_small(ase[rows])
        selq = np.zeros((G, S_SUB * P), F32)
        for s in range(S_SUB):
            pcol = np.arange(P)
            gsel = c * GPC + 2 * s + (pcol >= G).astype(np.int64)
            selq[gsel, s * P + pcol] = 1.0
        in_maps.append(dict(
            xarg_a=np.ascontiguousarray(xarg_a),
            w_a=np.ascontiguousarray(w_a),
            bpack=np.ascontiguousarray(bpack),
            spack=np.ascontiguousarray(spack),
            selq=np.ascontiguousarray(selq),
        ))
    return in_maps


def _conforming(batch, edge_index, edge_length, node_attrs, kappa,
                short_energy, atomic_short_energy, node_feats, **kw):
    if batch.shape != (T,) or node_attrs.shape != (T, 4):
        return False
    if short_energy.shape != (G, 1) or node_feats.shape[0] != T:
        return False
    if not np.array_equal(np.asarray(batch, np.int64),
                          np.repeat(np.arange(G, dtype=np.int64), N)):
        return False
    e0 = np.asarray(edge_index[0], np.int64)
    e1 = np.asarray(edge_index[1], np.int64)
    if e0.min() < 0 or e0.max() >= T or e1.min() < 0 or e1.max() >= T:
        return False
    if not np.all(e0 // N == e1 // N):          # all edges within a graph
        return False
    el = np.asarray(edge_length, F32)
    if not np.all((el > 0) & (el <= R_MAX)):    # keeps F_cut branch-free
        return False
    return True


def kernel(**inputs):
    import concourse.bass_utils as bass_utils

    args = {k: np.asarray(v) for k, v in inputs.items()}
    if not _conforming(**args):
        return _fallback_numpy(**args)

    nc = _get_compiled()
    in_maps = _host_prep(**args)
    res = bass_utils.run_bass_kernel_spmd(nc, in_maps,
                                          core_ids=list(range(NCORES)))

    charges = np.ascontiguousarray(
        res.results[0]["q_out"].reshape(T)).astype(F32)
    pot = np.empty(T, F32)
    ne = np.empty(T, F32)
    E_el = np.empty(G, F32)
    E2b_raw = np.empty(G, F32)
    for c in range(NCORES):
        r = res.results[c]
        rows = slice(c * APC, (c + 1) * APC)
        pot[rows] = r["pot_out"].T.reshape(APC)      # atom = s*128 + p
        ne[rows] = r["ne_out"].T.reshape(APC)
        sums = r["sums_out"]                         # [2, 8]
        for s in range(S_SUB):
            for h in range(2):
                g = c * GPC + 2 * s + h
                E_el[g] = sums[h, s]
                E2b_raw[g] = sums[h, 4 + s]

    E_tot = (E_el + F32(0.5) * E2b_raw).reshape(G, 1) \
        + args["short_energy"].astype(F32)
    node_energy = ne[:, None]
    node_feats_out = np.concatenate(
        [args["node_feats"].astype(F32), charges[:, None], pot[:, None]],
        axis=1).astype(F32)
    return (E_tot.astype(F32), node_energy.astype(F32), charges,
            node_feats_out)


# revision 20
# speedup vs baseline: 1.3278x; 1.3278x over previous
"""Trainium2 Bass kernel for the MACE charge-equilibration module (nn_MACE_44435731645168).

Mathematical restructuring (exact, validated against the jax reference):
  * batch = repeat(arange(64), 64) and every edge connects atoms of the same
    graph, so the dense TxT distance matrix is EPS everywhere except inside the
    64 diagonal 64x64 blocks.  Off-block entries of `factor` and `F_cut` are the
    analytic constants c_off = erf(1/sqrt(2))/EPS and c1 = tanh(1)^3, and the
    off-block part of each dense matvec collapses to c_off*(S_tot - S_g) with
    S_g the per-graph charge sum.  All remaining work is per-graph 64x64 blocks.
  * The extended (N+1)x(N+1) solve [[L,1],[1^T,0]] [q;lam] = [-kappa;0] with L
    lower-triangular reduces to two triangular solves: u = L^-1 kappa,
    v = L^-1 1, q = -u + (sum(u)/sum(v)) v.  Both are done for all 64 graphs at
    once on-device: graphs x {u,v} occupy the 128 SBUF partitions and each
    forward-substitution step is a single fused multiply-reduce instruction.

Device work: erf/tanh/exp transcendentals for all pair blocks, the 64-step
batched triangular solve, the blocked matvecs/reductions, and the PE matmuls
that shuffle charges between layouts.  Host work: dtype/layout prep, scattering
the edge list into dense 64x64 blocks (last-write-wins, matching XLA scatter),
and expanding the tiny 4x4 type-pair parameter tables.

Each of the 8 cores runs an identical program; per-core inputs differ only in
which 8 graphs (512 atoms) the core computes outputs for.  The (tiny) solve is
replicated on every core, which removes all cross-core communication.
"""

import math

import numpy as np

try:
    from scipy.special import erf as _erf
except Exception:  # pragma: no cover
    _erf = np.vectorize(math.erf, otypes=[np.float64])

G = 64
N = 64
T = G * N
P = 128
NCORES = 8
GPC = G // NCORES        # graphs per core
APC = T // NCORES        # atoms per core
S_SUB = APC // P         # free-dim sub-blocks in the per-atom layout (4)
EPS = np.float32(0.5)
R_MAX = np.float32(6.0)
R_IN = np.float32(1.0)
SQRT_PI = np.float32(np.sqrt(np.pi))
C1 = np.float32(np.tanh(1.0) ** 3)
C_OFF = np.float32(_erf(1.0 / np.sqrt(2.0)) / 0.5)

F32 = np.float32

# bpack free-dim slices ([128, 256] each)
BW = S_SUB * N  # 256
_BP_NAMES = ["xargB", "invRB", "tharg", "earg", "cd", "diag01"]
BP = BW * len(_BP_NAMES)
# spack free-dim layout: shift_sel [0:64], halves [64:66], dterm [66:70], ase [70:74]
SP = 80
SEL0 = BP + SP          # selq offset inside aux pack
AUXW = SEL0 + S_SUB * P  # aux pack width

# packed lower-triangle A layout: row j occupies [tri(j), tri(j)+j+1) with
# slot 0 carrying b_tilde and slots 1..j carrying -L[j,k]/L[j,j], k<j
TRI_OFF = np.cumsum([0] + [j + 1 for j in range(N)])  # [65]; TRI_OFF[64]=2080
TRI = int(TRI_OFF[N])
_RIDX = np.repeat(np.arange(N), np.arange(1, N + 1))          # [TRI]
_CIDX = np.concatenate([np.arange(j + 1) for j in range(N)])  # [TRI]

_compiled = None


def _a_chunks(nch):
    """Split packed rows 0..63 into ~nch chunks balanced by element count."""
    bounds = [0]
    target = TRI / nch
    for j in range(1, N):
        if TRI_OFF[j] >= target * len(bounds) and bounds[-1] != j:
            bounds.append(j)
    bounds.append(N)
    return [(bounds[i], bounds[i + 1]) for i in range(len(bounds) - 1)]


def _build():
    """Build + compile the Bass program once. Returns (nc, meta)."""
    import concourse.bacc as bacc
    import concourse.tile as tile
    import concourse.mybir as mybir
    from concourse import bass

    dt = mybir.dt.float32
    Act = mybir.ActivationFunctionType
    Op = mybir.AluOpType
    Ax = mybir.AxisListType

    nc = bacc.Bacc("TRN2", target_bir_lowering=False, debug=False,
                   num_devices=NCORES)

    xw_a = nc.dram_tensor("xw_a", [P, 2 * TRI], dt, kind="ExternalInput").ap()
    aux = nc.dram_tensor("aux", [P, AUXW], dt, kind="ExternalInput").ap()

    q_out = nc.dram_tensor("q_out", [G, N], dt, kind="ExternalOutput").ap()
    potne_out = nc.dram_tensor("potne_out", [P, 2 * S_SUB], dt,
                               kind="ExternalOutput").ap()
    sums_out = nc.dram_tensor("sums_out", [2, 8], dt, kind="ExternalOutput").ap()

    chunks = _a_chunks(6)   # packed A-stage row-group chunks

    def bslice(tile_ap, name):
        i = _BP_NAMES.index(name)
        return tile_ap[:, i * BW:(i + 1) * BW]

    with tile.TileContext(nc) as tc:
        with tc.tile_pool(name="main", bufs=1) as pool, \
             tc.tile_pool(name="chunks", bufs=3) as cpool, \
             tc.tile_pool(name="psum", bufs=2, space="PSUM") as pp:

            # ---------------- A stage: packed Lneg chunks ----------------
            # Packed rows: row j at [tri(j), tri(j)+j+1); slot 0 carries
            # b_tilde (via w = b_tilde/erf(1)), slots 1..j carry
            # -L[j,k]/L[j,j] for k<j.  One combined [xarg|w] DMA per chunk;
            # erf on Scalar, Lneg multiply on GpSimd so the Vector engine
            # stays free for the serial substitution chain.
            lneg = {}
            for (r0, r1) in chunks:
                o0, o1 = int(TRI_OFF[r0]), int(TRI_OFF[r1])
                cw = o1 - o0
                xw = cpool.tile([P, 2 * cw], dt, tag="xw")
                nc.sync.dma_start(out=xw[:], in_=xw_a[:, 2 * o0:2 * o1])
                ea = cpool.tile([P, cw], dt, tag="ea")
                nc.scalar.activation(out=ea[:], in_=xw[:, 0:cw], func=Act.Erf)
                ln = pool.tile([P, cw], dt, tag=f"lneg{r0}")
                nc.gpsimd.tensor_tensor(out=ln[:], in0=ea[:],
                                        in1=xw[:, cw:2 * cw], op=Op.mult)
                for j in range(r0, r1):
                    lneg[j] = (ln, int(TRI_OFF[j]) - o0)

            # ---------------- aux pack DMA (after solve-critical ones) ----
            auxt = pool.tile([P, AUXW], dt)
            nc.sync.dma_start(out=auxt[:], in_=aux[:])
            bp = auxt[:, 0:BP]
            sp = auxt[:, BP:BP + SP]
            sel = auxt[0:G, SEL0:SEL0 + S_SUB * P]

            # ---------------- batched forward substitution ----------------
            # partitions = graph g (rows 0-63, RHS u) and g+64 (RHS v)
            # y layout: col 0 = constant 1, cols 1..64 = solution, col 65 = sum
            y = pool.tile([P, N + 2], dt)
            scratch = pool.tile([P, N], dt)
            nc.vector.memset(y[:, 0:1], 1.0)
            for j in range(N):
                ln, off = lneg[j]
                nc.vector.scalar_tensor_tensor(
                    out=scratch[:, 0:j + 1],
                    in0=ln[:, off:off + j + 1],
                    scalar=1.0,
                    in1=y[:, 0:j + 1],
                    op0=Op.bypass,
                    op1=Op.mult,
                    accum_out=y[:, j + 1:j + 2],
                )
            # row sums -> y[:, 65]  (Sum u per graph / Sum v per graph)
            nc.vector.tensor_reduce(out=y[:, N + 1:N + 2], in_=y[:, 1:N + 1],
                                    axis=Ax.X, op=Op.add)

            # ---------------- q = -u + (Su/Sv) v ----------------
            vsh = pp.tile([G, N + 1], dt)
            nc.tensor.matmul(vsh[:], lhsT=sp[:, 0:64], rhs=y[:, 1:N + 2],
                             start=True, stop=True)
            rv = pool.tile([G, 1], dt)
            nc.vector.reciprocal(out=rv[:], in_=vsh[:, N:N + 1])
            r = pool.tile([G, 1], dt)
            nc.vector.tensor_tensor(out=r[:], in0=y[0:G, N + 1:N + 2], in1=rv[:],
                                    op=Op.mult)
            # qrhs: cols 0..63 = charges per graph, col 64 = S_tot - S_g
            qrhs = pool.tile([G, N + 1], dt)
            qa = qrhs[:, 0:N]
            nc.vector.scalar_tensor_tensor(out=qa, in0=vsh[:, 0:N],
                                           scalar=r[:], in1=y[0:G, 1:N + 1],
                                           op0=Op.mult, op1=Op.subtract)
            nc.sync.dma_start(out=q_out[:], in_=qa)

            # S_tot broadcast, then offw = S_tot - S_g into qrhs col 64
            sga = pool.tile([G, 1], dt)
            nc.vector.tensor_reduce(out=sga[:], in_=qa, axis=Ax.X, op=Op.add)
            ones64 = pool.tile([G, 1], dt)
            nc.vector.memset(ones64[:], 1.0)
            stp = pp.tile([1, 1], dt)
            nc.tensor.matmul(stp[:], lhsT=sga[:], rhs=ones64[:],
                             start=True, stop=True)
            sts = pool.tile([1, 1], dt)
            nc.vector.tensor_copy(out=sts[:], in_=stp[:])
            stb = pool.tile([G, 1], dt)
            nc.gpsimd.partition_broadcast(stb[:], sts[:])
            nc.vector.tensor_scalar(out=qrhs[:, N:N + 1], in0=sga[:],
                                    scalar1=stb[:], scalar2=-1.0,
                                    op0=Op.subtract, op1=Op.mult)

            # ---------------- charges + offw in per-atom layout ----------
            W1 = N + 1
            qbp = pp.tile([P, S_SUB * W1], dt)
            for s in range(S_SUB):
                nc.tensor.matmul(qbp[:, s * W1:(s + 1) * W1],
                                 lhsT=sel[:, s * P:(s + 1) * P], rhs=qrhs[:],
                                 start=True, stop=True)
            qbs = pool.tile([P, S_SUB * W1], dt)
            nc.vector.tensor_copy(out=qbs[:], in_=qbp[:])
            qb3 = qbs[:].rearrange("p (s w) -> p s w", w=W1)
            qB = qb3[:, :, 0:N]                   # [128, 4, 64] charges
            offw = qb3[:, :, N:N + 1]             # [128, 4, 1]  S_tot - S_g

            # ---------------- B stage (per-atom rows, this core's atoms) ----
            eb = pool.tile([P, BW], dt)
            nc.scalar.activation(out=eb[:], in_=bslice(bp, "xargB"), func=Act.Erf)
            fac = pool.tile([P, BW], dt)
            nc.gpsimd.tensor_tensor(out=fac[:], in0=eb[:],
                                    in1=bslice(bp, "invRB"), op=Op.mult)
            th = pool.tile([P, BW], dt)
            nc.scalar.activation(out=th[:], in_=bslice(bp, "tharg"), func=Act.Tanh)
            th2 = pool.tile([P, BW], dt)
            nc.gpsimd.tensor_tensor(out=th2[:], in0=th[:], in1=th[:], op=Op.mult)
            fcut = pool.tile([P, BW], dt)
            nc.gpsimd.tensor_tensor(out=fcut[:], in0=th2[:], in1=th[:], op=Op.mult)
            ex = pool.tile([P, BW], dt)
            nc.scalar.activation(out=ex[:], in_=bslice(bp, "earg"), func=Act.Exp)
            t2b = pool.tile([P, BW], dt)
            nc.gpsimd.tensor_tensor(out=t2b[:], in0=ex[:], in1=bslice(bp, "cd"),
                                    op=Op.subtract)
            t2b2 = pool.tile([P, BW], dt)
            nc.gpsimd.tensor_tensor(out=t2b2[:], in0=t2b[:], in1=fcut[:],
                                    op=Op.mult)

            sums_in = pool.tile([P, 8], dt)
            e2brow = sums_in[:, 4:8]
            nc.vector.tensor_reduce(
                out=e2brow, in_=t2b2[:].rearrange("p (s l) -> p s l", l=N),
                axis=Ax.X, op=Op.add)

            # matvecs: V_pre = sum(fac*q), P = sum(fac*q*Fc) per atom row
            tmpv = pool.tile([P, BW], dt)
            tv3 = tmpv[:].rearrange("p (s l) -> p s l", l=N)
            nc.vector.tensor_tensor(out=tv3, in0=fac[:].rearrange(
                "p (s l) -> p s l", l=N), in1=qB, op=Op.mult)
            vpre = pool.tile([P, S_SUB], dt)
            nc.vector.tensor_reduce(out=vpre[:], in_=tv3, axis=Ax.X, op=Op.add)
            tmpp = pool.tile([P, BW], dt)
            nc.vector.tensor_tensor(out=tmpp[:], in0=tmpv[:], in1=fcut[:],
                                    op=Op.mult)
            pb = pool.tile([P, S_SUB], dt)
            nc.vector.tensor_reduce(
                out=pb[:], in_=tmpp[:].rearrange("p (s l) -> p s l", l=N),
                axis=Ax.X, op=Op.add)

            # own-charge per atom via diagonal mask
            qd = pool.tile([P, BW], dt)
            nc.vector.tensor_tensor(out=qd[:].rearrange("p (s l) -> p s l", l=N),
                                    in0=qB, in1=bslice(bp, "diag01").rearrange(
                                        "p (s l) -> p s l", l=N), op=Op.mult)
            qown = pool.tile([P, S_SUB], dt)
            nc.vector.tensor_reduce(
                out=qown[:], in_=qd[:].rearrange("p (s l) -> p s l", l=N),
                axis=Ax.X, op=Op.add)

            # atomic potentials: P + c1*c_off*(S_tot - S_g)
            potne = pool.tile([P, 2 * S_SUB], dt)
            pot = potne[:, 0:S_SUB]
            nc.vector.scalar_tensor_tensor(
                out=pot.unsqueeze(-1), in0=offw, scalar=float(C1 * C_OFF),
                in1=pb[:].unsqueeze(-1), op0=Op.mult, op1=Op.add)

            # V, atomic electrostatic energy
            v1 = pool.tile([P, S_SUB], dt)
            nc.vector.scalar_tensor_tensor(
                out=v1[:].unsqueeze(-1), in0=offw, scalar=float(C_OFF),
                in1=vpre[:].unsqueeze(-1), op0=Op.mult, op1=Op.add)
            dq = pool.tile([P, S_SUB], dt)
            nc.vector.tensor_tensor(out=dq[:], in0=qown[:], in1=sp[:, 66:70],
                                    op=Op.mult)
            vfin = pool.tile([P, S_SUB], dt)
            nc.vector.tensor_tensor(out=vfin[:], in0=v1[:], in1=dq[:], op=Op.add)
            aeel = sums_in[:, 0:4]
            nc.vector.tensor_tensor(out=aeel, in0=qown[:], in1=vfin[:], op=Op.mult)

            ne = pool.tile([P, S_SUB], dt)
            nc.vector.scalar_tensor_tensor(out=ne[:], in0=e2brow, scalar=0.5,
                                           in1=aeel, op0=Op.mult, op1=Op.add)
            ne2 = potne[:, S_SUB:2 * S_SUB]
            nc.vector.tensor_tensor(out=ne2, in0=ne[:], in1=sp[:, 70:74],
                                    op=Op.add)
            nc.sync.dma_start(out=potne_out[:], in_=potne[:])

            # per-graph sums of aeel / e2brow  (halves selector matmul)
            smp = pp.tile([2, 8], dt)
            nc.tensor.matmul(smp[:], lhsT=sp[:, 64:66], rhs=sums_in[:],
                             start=True, stop=True)
            sms = pool.tile([2, 8], dt)
            nc.vector.tensor_copy(out=sms[:], in_=smp[:])
            nc.sync.dma_start(out=sums_out[:], in_=sms[:])

    nc.compile()
    return nc


def _get_compiled():
    global _compiled
    if _compiled is None:
        _compiled = _build()
    return _compiled


def _f_cut_np(R):
    c1 = np.float32(np.tanh(1.0) ** 3)
    smooth = np.tanh((1.0 - (R - R_IN) / (R_MAX - R_IN)).astype(F32)).astype(F32) ** 3
    out = np.where((R > 0) & (R < R_IN), c1,
                   np.where((R >= R_IN) & (R <= R_MAX), smooth, F32(0.0)))
    return out.astype(F32)


def _fallback_numpy(batch, node_attrs, edge_index, edge_length, kappa,
                    node_feats, ref_eta, ref_log_sigma, ref_A, ref_B, ref_C,
                    ref_D, ref_mu, short_energy, atomic_short_energy):
    """Faithful numpy port of the reference for non-conforming inputs."""
    Tn = batch.shape[0]
    Gn = int(batch.max()) + 1 if Tn else 0
    Gn = max(Gn, short_energy.shape[0])
    Nn = Tn // Gn
    types = np.argmax(node_attrs, axis=1)
    ref_sigma = np.exp(ref_log_sigma).astype(F32)
    sigma = ref_sigma[types]
    eta = ref_eta[types]
    ref_gamma = np.sqrt(ref_sigma[:, None] ** 2 + ref_sigma[None, :] ** 2).astype(F32)
    same = batch[:, None] == batch[None, :]
    gamma = np.where(same, ref_gamma[types[:, None], types[None, :]], EPS).astype(F32)
    R = np.full((Tn, Tn), EPS, dtype=F32)
    R[edge_index[0], edge_index[1]] = edge_length[:, 0]
    Fc = _f_cut_np(R)
    factor = (_erf((R / (np.sqrt(F32(2.0)) * gamma)).astype(F32)).astype(F32) / R).astype(F32)
    A_ij = np.diag((eta + 1.0 / (sigma * SQRT_PI)).astype(F32)) + factor
    A_tril = np.tril(A_ij).astype(F32)
    A4 = A_tril.reshape(Gn, Nn, Gn, Nn)
    gi = np.arange(Gn)
    A_blocks = A4[gi, :, gi, :]
    A_ext = np.zeros((Gn, Nn + 1, Nn + 1), dtype=F32)
    A_ext[:, :Nn, :Nn] = A_blocks
    A_ext[:, Nn, :Nn] = 1.0
    A_ext[:, :Nn, Nn] = 1.0
    kappa_ext = np.concatenate([kappa.reshape(Gn, Nn),
                                np.zeros((Gn, 1), F32)], axis=1)
    sol = np.linalg.solve(A_ext.astype(np.float64),
                          -kappa_ext[..., None].astype(np.float64))[..., 0]
    charges = sol[:, :Nn].reshape(Tn).astype(F32)
    atomic_potentials = ((factor * Fc) @ charges).astype(F32)
    E_factor = np.diag((0.5 / (sigma * SQRT_PI)).astype(F32)) + factor
    V = (E_factor @ charges).astype(F32)
    atomic_E_el = (charges * V).astype(F32)
    E_el = np.zeros((Gn, 1), F32)
    np.add.at(E_el[:, 0], batch, atomic_E_el)
    pair = lambda ref: np.where(same, ref[types[:, None], types[None, :]], F32(0.0)).astype(F32)
    Am, Bm, Cm, Dm, mum = (pair(x) for x in (ref_A, ref_B, ref_C, ref_D, ref_mu))
    E2b_ij = ((Am * np.exp((Bm * (mum - R)).astype(F32)).astype(F32)
               - Cm / R ** 6 - Dm / R ** 8).astype(F32) * Fc)
    E2b_ij *= (1.0 - np.eye(Tn, dtype=F32))
    atomic_E2b = (E2b_ij.sum(axis=-1) * 0.5)[:, None].astype(F32)
    E_2b = np.zeros((Gn, 1), F32)
    np.add.at(E_2b[:, 0], batch, atomic_E2b[:, 0])
    E_tot = (E_el + E_2b + short_energy.astype(F32)).astype(F32)
    node_energy = (atomic_E_el[:, None] + atomic_E2b
                   + atomic_short_energy.astype(F32)).astype(F32)
    node_feats_out = np.concatenate(
        [node_feats.astype(F32), charges[:, None], atomic_potentials[:, None]],
        axis=1).astype(F32)
    return E_tot, node_energy, charges, node_feats_out


def _host_prep(batch, node_attrs, edge_index, edge_length, kappa, node_feats,
               ref_eta, ref_log_sigma, ref_A, ref_B, ref_C, ref_D, ref_mu,
               short_energy, atomic_short_energy):
    """Numpy-side layout/table prep. Returns per-core in_maps."""
    types = np.argmax(node_attrs, axis=1).astype(np.int64)            # [T]
    ref_sigma = np.exp(ref_log_sigma.astype(F32)).astype(F32)
    sigma = ref_sigma[types]
    eta = ref_eta.astype(F32)[types]
    ref_gamma = np.sqrt(ref_sigma[:, None] ** 2
                        + ref_sigma[None, :] ** 2).astype(F32)        # [4,4]

    # dense per-graph blocks of R (last-write-wins scatter, matches XLA)
    e0 = edge_index[0].astype(np.int64)
    e1 = edge_index[1].astype(np.int64)
    Rb = np.full((G, N, N), EPS, dtype=F32)
    Rb[e0 // N, e0 % N, e1 % N] = edge_length[:, 0].astype(F32)

    tb = types.reshape(G, N)                                          # [G,N]
    gam = ref_gamma[tb[:, :, None], tb[:, None, :]]                   # [G,N,N]
    inv_sqrt2g = (F32(1.0) / (np.sqrt(F32(2.0)) * gam)).astype(F32)
    xargA = (Rb * inv_sqrt2g).astype(F32)                             # [G,N,N]
    invR = (F32(1.0) / Rb).astype(F32)

    # diagonal of L and the row pre-scaling
    idx = np.arange(N)
    fdiag = (_erf(xargA[:, idx, idx]).astype(F32) * invR[:, idx, idx]).astype(F32)
    dvec = (eta + F32(1.0) / (sigma * SQRT_PI)).astype(F32).reshape(G, N)
    invd = (F32(1.0) / (dvec + fdiag)).astype(F32)                    # [G,N]
    wA = (-invd[:, :, None] * invR).astype(F32)                       # [G,N,N]

    # packed lower-triangle rows; slot 0 of row j carries b_tilde via
    # erf(1.0) * (b_tilde/erf(1.0)) = b_tilde
    erf1 = F32(_erf(np.float32(1.0)))
    bu = (invd * kappa.astype(F32).reshape(G, N) / erf1).astype(F32)
    bv = (invd / erf1).astype(F32)
    cid0 = np.maximum(_CIDX - 1, 0)
    half_x = np.where(_CIDX == 0, F32(1.0), xargA[:, _RIDX, cid0])    # [G,TRI]
    wgat = wA[:, _RIDX, cid0]
    w_u = np.where(_CIDX == 0, bu[:, _RIDX], wgat).astype(F32)
    w_v = np.where(_CIDX == 0, bv[:, _RIDX], wgat).astype(F32)
    xarg_a = np.concatenate([half_x, half_x], axis=0).astype(F32)     # [128,TRI]
    w_a = np.concatenate([w_u, w_v], axis=0)                          # [128,TRI]
    # combined [xarg | w] per chunk, chunks back to back
    xw_a = np.empty((P, 2 * TRI), F32)
    for (r0, r1) in _a_chunks(6):
        o0, o1 = int(TRI_OFF[r0]), int(TRI_OFF[r1])
        xw_a[:, 2 * o0:o0 + o1] = xarg_a[:, o0:o1]
        xw_a[:, o0 + o1:2 * o1] = w_a[:, o0:o1]

    # ---- per-atom (B) layout, full problem then sliced per core ----
    gi_of = np.arange(T) // N
    li_of = np.arange(T) % N
    R_row = Rb[gi_of, li_of, :]                                       # [T,64]
    xargB = xargA[gi_of, li_of, :]
    invRB = invR[gi_of, li_of, :]
    tharg = (F32(1.2) - F32(0.2) * np.clip(R_row, R_IN, R_MAX)).astype(F32)

    t_i = types[:, None]                                              # [T,1]
    t_j = tb[gi_of]                                                   # [T,64]
    lnA = np.log(ref_A.astype(F32)).astype(F32)
    beta = ref_B.astype(F32)[t_i, t_j]
    delta = (beta * ref_mu.astype(F32)[t_i, t_j] + lnA[t_i, t_j]).astype(F32)
    earg = (delta - beta * R_row).astype(F32)
    iR2 = (invRB * invRB).astype(F32)
    iR6 = (iR2 * iR2 * iR2).astype(F32)
    cd = (ref_C.astype(F32)[t_i, t_j] * iR6
          + ref_D.astype(F32)[t_i, t_j] * iR6 * iR2).astype(F32)
    diag01 = (t_j * 0).astype(F32)
    diag01[np.arange(T), li_of] = 1.0
    earg[np.arange(T), li_of] = -100.0
    cd[np.arange(T), li_of] = 0.0

    dterm = (F32(0.5) / (sigma * SQRT_PI)).astype(F32)                # [T]
    ase = atomic_short_energy.astype(F32)[:, 0]                       # [T]

    # constant small tensors
    shift_sel = np.zeros((P, G), F32)
    shift_sel[np.arange(G) + G, np.arange(G)] = 1.0                   # k==m+64
    halves = np.zeros((P, 2), F32)
    halves[:G, 0] = 1.0
    halves[G:, 1] = 1.0

    def to_b_layout(arr_tc):  # [APC, 64] -> [128, 256], atom = s*128 + p
        return np.ascontiguousarray(
            arr_tc.reshape(S_SUB, P, N).transpose(1, 0, 2).reshape(P, BW))

    def to_b_small(vec):      # [APC] -> [128, 4]
        return np.ascontiguousarray(vec.reshape(S_SUB, P).T)

    in_maps = []
    for c in range(NCORES):
        rows = slice(c * APC, (c + 1) * APC)
        auxp = np.zeros((P, AUXW), F32)
        auxp[:, 0:BP] = np.concatenate(
            [to_b_layout(a[rows]) for a in (xargB, invRB, tharg, earg, cd, diag01)],
            axis=1)
        auxp[:, BP + 0:BP + 64] = shift_sel
        auxp[:, BP + 64:BP + 66] = halves
        auxp[:, BP + 66:BP + 70] = to_b_small(dterm[rows])
        auxp[:, BP + 70:BP + 74] = to_b_small(ase[rows])
        for s in range(S_SUB):
            pcol = np.arange(P)
            gsel = c * GPC + 2 * s + (pcol >= G).astype(np.int64)
            auxp[gsel, SEL0 + s * P + pcol] = 1.0
        in_maps.append(dict(
            xw_a=np.ascontiguousarray(xw_a),
            aux=np.ascontiguousarray(auxp),
        ))
    return in_maps


def _conforming(batch, edge_index, edge_length, node_attrs, kappa,
                short_energy, atomic_short_energy, node_feats, **kw):
    if batch.shape != (T,) or node_attrs.shape != (T, 4):
        return False
    if short_energy.shape != (G, 1) or node_feats.shape[0] != T:
        return False
    if not np.array_equal(np.asarray(batch, np.int64),
                          np.repeat(np.arange(G, dtype=np.int64), N)):
        return False
    e0 = np.asarray(edge_index[0], np.int64)
    e1 = np.asarray(edge_index[1], np.int64)
    if e0.min() < 0 or e0.max() >= T or e1.min() < 0 or e1.max() >= T:
        return False
    if not np.all(e0 // N == e1 // N):          # all edges within a graph
        return False
    el = np.asarray(edge_length, F32)
    if not np.all((el > 0) & (el <= R_MAX)):    # keeps F_cut branch-free
        return False
    return True


def kernel(**inputs):
    import concourse.bass_utils as bass_utils

    args = {k: np.asarray(v) for k, v in inputs.items()}
    if not _conforming(**args):
        return _fallback_numpy(**args)

    nc = _get_compiled()
    in_maps = _host_prep(**args)
    res = bass_utils.run_bass_kernel_spmd(nc, in_maps,
                                          core_ids=list(range(NCORES)))

    charges = np.ascontiguousarray(
        res.results[0]["q_out"].reshape(T)).astype(F32)
    pot = np.empty(T, F32)
    ne = np.empty(T, F32)
    E_el = np.empty(G, F32)
    E2b_raw = np.empty(G, F32)
    for c in range(NCORES):
        r = res.results[c]
        rows = slice(c * APC, (c + 1) * APC)
        pn = r["potne_out"]                          # [128, 8]
        pot[rows] = pn[:, 0:S_SUB].T.reshape(APC)    # atom = s*128 + p
        ne[rows] = pn[:, S_SUB:2 * S_SUB].T.reshape(APC)
        sums = r["sums_out"]                         # [2, 8]
        for s in range(S_SUB):
            for h in range(2):
                g = c * GPC + 2 * s + h
                E_el[g] = sums[h, s]
                E2b_raw[g] = sums[h, 4 + s]

    E_tot = (E_el + F32(0.5) * E2b_raw).reshape(G, 1) \
        + args["short_energy"].astype(F32)
    node_energy = ne[:, None]
    node_feats_out = np.concatenate(
        [args["node_feats"].astype(F32), charges[:, None], pot[:, None]],
        axis=1).astype(F32)
    return (E_tot.astype(F32), node_energy.astype(F32), charges,
            node_feats_out)


# revision 26
# speedup vs baseline: 1.3441x; 1.0123x over previous
"""Trainium2 Bass kernel for the MACE charge-equilibration module (nn_MACE_44435731645168).

Mathematical restructuring (exact, validated against the jax reference):
  * batch = repeat(arange(64), 64) and every edge connects atoms of the same
    graph, so the dense TxT distance matrix is EPS everywhere except inside the
    64 diagonal 64x64 blocks.  Off-block entries of `factor` and `F_cut` are the
    analytic constants c_off = erf(1/sqrt(2))/EPS and c1 = tanh(1)^3, and the
    off-block part of each dense matvec collapses to c_off*(S_tot - S_g) with
    S_g the per-graph charge sum.  All remaining work is per-graph 64x64 blocks.
  * The extended (N+1)x(N+1) solve [[L,1],[1^T,0]] [q;lam] = [-kappa;0] with L
    lower-triangular reduces to two triangular solves: u = L^-1 kappa,
    v = L^-1 1, q = -u + (sum(u)/sum(v)) v.  Both are done for all 64 graphs at
    once on-device: graphs x {u,v} occupy the 128 SBUF partitions and each
    forward-substitution step is a single fused multiply-reduce instruction.

Device work: erf/tanh/exp transcendentals for all pair blocks, the 64-step
batched triangular solve, the blocked matvecs/reductions, and the PE matmuls
that shuffle charges between layouts.  Host work: dtype/layout prep, scattering
the edge list into dense 64x64 blocks (last-write-wins, matching XLA scatter),
and expanding the tiny 4x4 type-pair parameter tables.

Each of the 8 cores runs an identical program; per-core inputs differ only in
which 8 graphs (512 atoms) the core computes outputs for.  The (tiny) solve is
replicated on every core, which removes all cross-core communication.
"""

import math

import numpy as np

try:
    from scipy.special import erf as _erf
except Exception:  # pragma: no cover
    _erf = np.vectorize(math.erf, otypes=[np.float64])

G = 64
N = 64
T = G * N
P = 128
NCORES = 8
GPC = G // NCORES        # graphs per core
APC = T // NCORES        # atoms per core
S_SUB = APC // P         # free-dim sub-blocks in the per-atom layout (4)
EPS = np.float32(0.5)
R_MAX = np.float32(6.0)
R_IN = np.float32(1.0)
SQRT_PI = np.float32(np.sqrt(np.pi))
C1 = np.float32(np.tanh(1.0) ** 3)
C_OFF = np.float32(_erf(1.0 / np.sqrt(2.0)) / 0.5)

F32 = np.float32

# bpack free-dim slices ([128, 256] each)
BW = S_SUB * N  # 256
_BP_NAMES = ["xargB", "invRB", "tharg", "earg", "cd", "diag01"]
BP = BW * len(_BP_NAMES)
# spack free-dim layout: shift_sel [0:64], halves [64:66], dterm [66:70], ase [70:74]
SP = 80
SEL0 = BP + SP          # selq offset inside aux pack
AUXW = SEL0 + S_SUB * P  # aux pack width

# packed lower-triangle A layout: row j occupies [tri(j), tri(j)+j+1) with
# slot 0 carrying b_tilde and slots 1..j carrying -L[j,k]/L[j,j], k<j
TRI_OFF = np.cumsum([0] + [j + 1 for j in range(N)])  # [65]; TRI_OFF[64]=2080
TRI = int(TRI_OFF[N])
_RIDX = np.repeat(np.arange(N), np.arange(1, N + 1))          # [TRI]
_CIDX = np.concatenate([np.arange(j + 1) for j in range(N)])  # [TRI]

_compiled = None


def _a_chunks(nch=None):
    """Packed-row chunks; first chunk small so the solve starts early."""
    bounds = [0, 16, 32, 48, N]
    return [(bounds[i], bounds[i + 1]) for i in range(len(bounds) - 1)]


def _build():
    """Build + compile the Bass program once. Returns (nc, meta)."""
    import concourse.bacc as bacc
    import concourse.tile as tile
    import concourse.mybir as mybir
    from concourse import bass

    dt = mybir.dt.float32
    Act = mybir.ActivationFunctionType
    Op = mybir.AluOpType
    Ax = mybir.AxisListType

    nc = bacc.Bacc("TRN2", target_bir_lowering=False, debug=False,
                   num_devices=NCORES)

    xw_a = nc.dram_tensor("xw_a", [P, 2 * TRI], dt, kind="ExternalInput").ap()
    aux = nc.dram_tensor("aux", [P, AUXW], dt, kind="ExternalInput").ap()

    q_out = nc.dram_tensor("q_out", [G, N], dt, kind="ExternalOutput").ap()
    potne_out = nc.dram_tensor("potne_out", [P, 2 * S_SUB], dt,
                               kind="ExternalOutput").ap()
    sums_out = nc.dram_tensor("sums_out", [2, 8], dt, kind="ExternalOutput").ap()

    from concourse.tile import add_dep_helper

    chunks = _a_chunks()    # packed A-stage row-group chunks

    def bslice(tile_ap, name):
        i = _BP_NAMES.index(name)
        return tile_ap[:, i * BW:(i + 1) * BW]

    with tile.TileContext(nc) as tc:
        with tc.tile_pool(name="main", bufs=1) as pool, \
             tc.tile_pool(name="chunks", bufs=3) as cpool, \
             tc.tile_pool(name="psum", bufs=2, space="PSUM") as pp:

            # ---------------- A stage: packed Lneg chunks ----------------
            # Packed rows: row j at [tri(j), tri(j)+j+1); slot 0 carries
            # b_tilde (via w = b_tilde/erf(1)), slots 1..j carry
            # -L[j,k]/L[j,j] for k<j.  One combined [xarg|w] DMA per chunk;
            # erf on Scalar, Lneg multiply on GpSimd so the Vector engine
            # stays free for the serial substitution chain.
            lneg = {}
            for (r0, r1) in chunks:
                o0, o1 = int(TRI_OFF[r0]), int(TRI_OFF[r1])
                cw = o1 - o0
                xw = cpool.tile([P, 2 * cw], dt, tag="xw")
                nc.sync.dma_start(out=xw[:], in_=xw_a[:, 2 * o0:2 * o1])
                ea = cpool.tile([P, cw], dt, tag="ea")
                nc.scalar.activation(out=ea[:], in_=xw[:, 0:cw], func=Act.Erf)
                ln = pool.tile([P, cw], dt, tag=f"lneg{r0}")
                nc.gpsimd.tensor_tensor(out=ln[:], in0=ea[:],
                                        in1=xw[:, cw:2 * cw], op=Op.mult)
                for j in range(r0, r1):
                    lneg[j] = (ln, int(TRI_OFF[j]) - o0)

            # ---------------- aux pack DMA (after solve-critical ones) ----
            auxt = pool.tile([P, AUXW], dt)
            nc.sync.dma_start(out=auxt[:], in_=aux[:])
            bp = auxt[:, 0:BP]
            sp = auxt[:, BP:BP + SP]
            sel = auxt[0:G, SEL0:SEL0 + S_SUB * P]

            # ---------------- batched forward substitution ----------------
            # partitions = graph g (rows 0-63, RHS u) and g+64 (RHS v)
            # y layout: col 0 = constant 1, cols 1..64 = solution, col 65 = sum
            y = pool.tile([P, N + 2], dt)
            scratch = pool.tile([P, N], dt)
            nc.vector.memset(y[:, 0:1], 1.0)
            solve_steps = []
            for j in range(N):
                ln, off = lneg[j]
                si = nc.vector.scalar_tensor_tensor(
                    out=scratch[:, 0:j + 1],
                    in0=ln[:, off:off + j + 1],
                    scalar=1.0,
                    in1=y[:, 0:j + 1],
                    op0=Op.bypass,
                    op1=Op.mult,
                    accum_out=y[:, j + 1:j + 2],
                )
                solve_steps.append(si)
            # row sums -> y[:, 65]  (Sum u per graph / Sum v per graph)
            nc.vector.tensor_reduce(out=y[:, N + 1:N + 2], in_=y[:, 1:N + 1],
                                    axis=Ax.X, op=Op.add)

            # ---------------- q = -u + (Su/Sv) v ----------------
            vsh = pp.tile([G, N + 1], dt)
            nc.tensor.matmul(vsh[:], lhsT=sp[:, 0:64], rhs=y[:, 1:N + 2],
                             start=True, stop=True)
            rv = pool.tile([G, 1], dt)
            nc.vector.reciprocal(out=rv[:], in_=vsh[:, N:N + 1])
            r = pool.tile([G, 1], dt)
            nc.vector.tensor_tensor(out=r[:], in0=y[0:G, N + 1:N + 2], in1=rv[:],
                                    op=Op.mult)
            # qrhs: cols 0..63 = charges per graph, col 64 = S_tot - S_g
            qrhs = pool.tile([G, N + 1], dt)
            qa = qrhs[:, 0:N]
            nc.vector.scalar_tensor_tensor(out=qa, in0=vsh[:, 0:N],
                                           scalar=r[:], in1=y[0:G, 1:N + 1],
                                           op0=Op.mult, op1=Op.subtract)
            nc.sync.dma_start(out=q_out[:], in_=qa)

            # S_tot broadcast, then offw = S_tot - S_g into qrhs col 64
            sga = pool.tile([G, 1], dt)
            nc.vector.tensor_reduce(out=sga[:], in_=qa, axis=Ax.X, op=Op.add)
            ones64 = pool.tile([G, 1], dt)
            nc.vector.memset(ones64[:], 1.0)
            stp = pp.tile([1, 1], dt)
            nc.tensor.matmul(stp[:], lhsT=sga[:], rhs=ones64[:],
                             start=True, stop=True)
            sts = pool.tile([1, 1], dt)
            nc.vector.tensor_copy(out=sts[:], in_=stp[:])
            stb = pool.tile([G, 1], dt)
            nc.gpsimd.partition_broadcast(stb[:], sts[:])
            nc.vector.tensor_scalar(out=qrhs[:, N:N + 1], in0=sga[:],
                                    scalar1=stb[:], scalar2=-1.0,
                                    op0=Op.subtract, op1=Op.mult)

            # ---------------- charges + offw in per-atom layout ----------
            W1 = N + 1
            qbp = pp.tile([P, S_SUB * W1], dt)
            for s in range(S_SUB):
                nc.tensor.matmul(qbp[:, s * W1:(s + 1) * W1],
                                 lhsT=sel[:, s * P:(s + 1) * P], rhs=qrhs[:],
                                 start=True, stop=True)
            qbs = pool.tile([P, S_SUB * W1], dt)
            nc.vector.tensor_copy(out=qbs[:], in_=qbp[:])
            qb3 = qbs[:].rearrange("p (s w) -> p s w", w=W1)
            qB = qb3[:, :, 0:N]                   # [128, 4, 64] charges
            offw = qb3[:, :, N:N + 1]             # [128, 4, 1]  S_tot - S_g

            # ---------------- B stage (per-atom rows, this core's atoms) ----
            eb = pool.tile([P, BW], dt)
            nc.scalar.activation(out=eb[:], in_=bslice(bp, "xargB"), func=Act.Erf)
            fac = pool.tile([P, BW], dt)
            nc.gpsimd.tensor_tensor(out=fac[:], in0=eb[:],
                                    in1=bslice(bp, "invRB"), op=Op.mult)
            th = pool.tile([P, BW], dt)
            nc.scalar.activation(out=th[:], in_=bslice(bp, "tharg"), func=Act.Tanh)
            th2 = pool.tile([P, BW], dt)
            nc.gpsimd.tensor_tensor(out=th2[:], in0=th[:], in1=th[:], op=Op.mult)
            fcut = pool.tile([P, BW], dt)
            nc.gpsimd.tensor_tensor(out=fcut[:], in0=th2[:], in1=th[:], op=Op.mult)
            ex = pool.tile([P, BW], dt)
            nc.scalar.activation(out=ex[:], in_=bslice(bp, "earg"), func=Act.Exp)
            t2b = pool.tile([P, BW], dt)
            i_t2b = nc.gpsimd.tensor_tensor(out=t2b[:], in0=ex[:],
                                            in1=bslice(bp, "cd"),
                                            op=Op.subtract)
            t2b2 = pool.tile([P, BW], dt)
            nc.gpsimd.tensor_tensor(out=t2b2[:], in0=t2b[:], in1=fcut[:],
                                    op=Op.mult)
            # keep the e2b chain off the SBUF ports while the serial
            # substitution runs (pure scheduling hint, no data dep)
            add_dep_helper(solve_steps[-1].ins, i_t2b.ins, sync=False,
                           reason="keep gpsimd off SBUF during solve")

            sums_in = pool.tile([P, 8], dt)
            e2brow = sums_in[:, 4:8]
            nc.vector.tensor_reduce(
                out=e2brow, in_=t2b2[:].rearrange("p (s l) -> p s l", l=N),
                axis=Ax.X, op=Op.add)

            # products tile: [tmpv | tmpp | qdiag], one grouped reduce
            prods = pool.tile([P, 3 * BW], dt)
            tmpv = prods[:, 0:BW]
            tv3 = tmpv.rearrange("p (s l) -> p s l", l=N)
            nc.vector.tensor_tensor(out=tv3, in0=fac[:].rearrange(
                "p (s l) -> p s l", l=N), in1=qB, op=Op.mult)
            nc.vector.tensor_tensor(out=prods[:, BW:2 * BW], in0=tmpv,
                                    in1=fcut[:], op=Op.mult)
            nc.vector.tensor_tensor(
                out=prods[:, 2 * BW:3 * BW].rearrange("p (s l) -> p s l", l=N),
                in0=qB, in1=bslice(bp, "diag01").rearrange(
                    "p (s l) -> p s l", l=N), op=Op.mult)
            red12 = pool.tile([P, 3 * S_SUB], dt)
            nc.vector.tensor_reduce(
                out=red12[:], in_=prods[:].rearrange("p (s l) -> p s l", l=N),
                axis=Ax.X, op=Op.add)
            vpre = red12[:, 0:S_SUB]
            pb = red12[:, S_SUB:2 * S_SUB]
            qown = red12[:, 2 * S_SUB:3 * S_SUB]

            # atomic potentials: P + c1*c_off*(S_tot - S_g)
            potne = pool.tile([P, 2 * S_SUB], dt)
            pot = potne[:, 0:S_SUB]
            nc.vector.scalar_tensor_tensor(
                out=pot.unsqueeze(-1), in0=offw, scalar=float(C1 * C_OFF),
                in1=pb.unsqueeze(-1), op0=Op.mult, op1=Op.add)

            # V, atomic electrostatic energy
            v1 = pool.tile([P, S_SUB], dt)
            nc.vector.scalar_tensor_tensor(
                out=v1[:].unsqueeze(-1), in0=offw, scalar=float(C_OFF),
                in1=vpre.unsqueeze(-1), op0=Op.mult, op1=Op.add)
            dq = pool.tile([P, S_SUB], dt)
            nc.vector.tensor_tensor(out=dq[:], in0=qown, in1=sp[:, 66:70],
                                    op=Op.mult)
            vfin = pool.tile([P, S_SUB], dt)
            nc.vector.tensor_tensor(out=vfin[:], in0=v1[:], in1=dq[:], op=Op.add)
            aeel = sums_in[:, 0:4]
            nc.vector.tensor_tensor(out=aeel, in0=qown, in1=vfin[:], op=Op.mult)

            ne = pool.tile([P, S_SUB], dt)
            nc.vector.scalar_tensor_tensor(out=ne[:], in0=e2brow, scalar=0.5,
                                           in1=aeel, op0=Op.mult, op1=Op.add)
            ne2 = potne[:, S_SUB:2 * S_SUB]
            nc.vector.tensor_tensor(out=ne2, in0=ne[:], in1=sp[:, 70:74],
                                    op=Op.add)
            nc.sync.dma_start(out=potne_out[:], in_=potne[:])

            # per-graph sums of aeel / e2brow  (halves selector matmul)
            smp = pp.tile([2, 8], dt)
            nc.tensor.matmul(smp[:], lhsT=sp[:, 64:66], rhs=sums_in[:],
                             start=True, stop=True)
            sms = pool.tile([2, 8], dt)
            nc.vector.tensor_copy(out=sms[:], in_=smp[:])
            nc.sync.dma_start(out=sums_out[:], in_=sms[:])

    nc.compile()
    return nc


def _get_compiled():
    global _compiled
    if _compiled is None:
        _compiled = _build()
    return _compiled


def _f_cut_np(R):
    c1 = np.float32(np.tanh(1.0) ** 3)
    smooth = np.tanh((1.0 - (R - R_IN) / (R_MAX - R_IN)).astype(F32)).astype(F32) ** 3
    out = np.where((R > 0) & (R < R_IN), c1,
                   np.where((R >= R_IN) & (R <= R_MAX), smooth, F32(0.0)))
    return out.astype(F32)


def _fallback_numpy(batch, node_attrs, edge_index, edge_length, kappa,
                    node_feats, ref_eta, ref_log_sigma, ref_A, ref_B, ref_C,
                    ref_D, ref_mu, short_energy, atomic_short_energy):
    """Faithful numpy port of the reference for non-conforming inputs."""
    Tn = batch.shape[0]
    Gn = int(batch.max()) + 1 if Tn else 0
    Gn = max(Gn, short_energy.shape[0])
    Nn = Tn // Gn
    types = np.argmax(node_attrs, axis=1)
    ref_sigma = np.exp(ref_log_sigma).astype(F32)
    sigma = ref_sigma[types]
    eta = ref_eta[types]
    ref_gamma = np.sqrt(ref_sigma[:, None] ** 2 + ref_sigma[None, :] ** 2).astype(F32)
    same = batch[:, None] == batch[None, :]
    gamma = np.where(same, ref_gamma[types[:, None], types[None, :]], EPS).astype(F32)
    R = np.full((Tn, Tn), EPS, dtype=F32)
    R[edge_index[0], edge_index[1]] = edge_length[:, 0]
    Fc = _f_cut_np(R)
    factor = (_erf((R / (np.sqrt(F32(2.0)) * gamma)).astype(F32)).astype(F32) / R).astype(F32)
    A_ij = np.diag((eta + 1.0 / (sigma * SQRT_PI)).astype(F32)) + factor
    A_tril = np.tril(A_ij).astype(F32)
    A4 = A_tril.reshape(Gn, Nn, Gn, Nn)
    gi = np.arange(Gn)
    A_blocks = A4[gi, :, gi, :]
    A_ext = np.zeros((Gn, Nn + 1, Nn + 1), dtype=F32)
    A_ext[:, :Nn, :Nn] = A_blocks
    A_ext[:, Nn, :Nn] = 1.0
    A_ext[:, :Nn, Nn] = 1.0
    kappa_ext = np.concatenate([kappa.reshape(Gn, Nn),
                                np.zeros((Gn, 1), F32)], axis=1)
    sol = np.linalg.solve(A_ext.astype(np.float64),
                          -kappa_ext[..., None].astype(np.float64))[..., 0]
    charges = sol[:, :Nn].reshape(Tn).astype(F32)
    atomic_potentials = ((factor * Fc) @ charges).astype(F32)
    E_factor = np.diag((0.5 / (sigma * SQRT_PI)).astype(F32)) + factor
    V = (E_factor @ charges).astype(F32)
    atomic_E_el = (charges * V).astype(F32)
    E_el = np.zeros((Gn, 1), F32)
    np.add.at(E_el[:, 0], batch, atomic_E_el)
    pair = lambda ref: np.where(same, ref[types[:, None], types[None, :]], F32(0.0)).astype(F32)
    Am, Bm, Cm, Dm, mum = (pair(x) for x in (ref_A, ref_B, ref_C, ref_D, ref_mu))
    E2b_ij = ((Am * np.exp((Bm * (mum - R)).astype(F32)).astype(F32)
               - Cm / R ** 6 - Dm / R ** 8).astype(F32) * Fc)
    E2b_ij *= (1.0 - np.eye(Tn, dtype=F32))
    atomic_E2b = (E2b_ij.sum(axis=-1) * 0.5)[:, None].astype(F32)
    E_2b = np.zeros((Gn, 1), F32)
    np.add.at(E_2b[:, 0], batch, atomic_E2b[:, 0])
    E_tot = (E_el + E_2b + short_energy.astype(F32)).astype(F32)
    node_energy = (atomic_E_el[:, None] + atomic_E2b
                   + atomic_short_energy.astype(F32)).astype(F32)
    node_feats_out = np.concatenate(
        [node_feats.astype(F32), charges[:, None], atomic_potentials[:, None]],
        axis=1).astype(F32)
    return E_tot, node_energy, charges, node_feats_out


def _host_prep(batch, node_attrs, edge_index, edge_length, kappa, node_feats,
               ref_eta, ref_log_sigma, ref_A, ref_B, ref_C, ref_D, ref_mu,
               short_energy, atomic_short_energy):
    """Numpy-side layout/table prep. Returns per-core in_maps."""
    types = np.argmax(node_attrs, axis=1).astype(np.int64)            # [T]
    ref_sigma = np.exp(ref_log_sigma.astype(F32)).astype(F32)
    sigma = ref_sigma[types]
    eta = ref_eta.astype(F32)[types]
    ref_gamma = np.sqrt(ref_sigma[:, None] ** 2
                        + ref_sigma[None, :] ** 2).astype(F32)        # [4,4]

    # dense per-graph blocks of R (last-write-wins scatter, matches XLA)
    e0 = edge_index[0].astype(np.int64)
    e1 = edge_index[1].astype(np.int64)
    Rb = np.full((G, N, N), EPS, dtype=F32)
    Rb[e0 // N, e0 % N, e1 % N] = edge_length[:, 0].astype(F32)

    tb = types.reshape(G, N)                                          # [G,N]
    gam = ref_gamma[tb[:, :, None], tb[:, None, :]]                   # [G,N,N]
    inv_sqrt2g = (F32(1.0) / (np.sqrt(F32(2.0)) * gam)).astype(F32)
    xargA = (Rb * inv_sqrt2g).astype(F32)                             # [G,N,N]
    invR = (F32(1.0) / Rb).astype(F32)

    # diagonal of L and the row pre-scaling
    idx = np.arange(N)
    fdiag = (_erf(xargA[:, idx, idx]).astype(F32) * invR[:, idx, idx]).astype(F32)
    dvec = (eta + F32(1.0) / (sigma * SQRT_PI)).astype(F32).reshape(G, N)
    invd = (F32(1.0) / (dvec + fdiag)).astype(F32)                    # [G,N]
    wA = (-invd[:, :, None] * invR).astype(F32)                       # [G,N,N]

    # packed lower-triangle rows; slot 0 of row j carries b_tilde via
    # erf(1.0) * (b_tilde/erf(1.0)) = b_tilde
    erf1 = F32(_erf(np.float32(1.0)))
    bu = (invd * kappa.astype(F32).reshape(G, N) / erf1).astype(F32)
    bv = (invd / erf1).astype(F32)
    cid0 = np.maximum(_CIDX - 1, 0)
    half_x = np.where(_CIDX == 0, F32(1.0), xargA[:, _RIDX, cid0])    # [G,TRI]
    wgat = wA[:, _RIDX, cid0]
    w_u = np.where(_CIDX == 0, bu[:, _RIDX], wgat).astype(F32)
    w_v = np.where(_CIDX == 0, bv[:, _RIDX], wgat).astype(F32)
    xarg_a = np.concatenate([half_x, half_x], axis=0).astype(F32)     # [128,TRI]
    w_a = np.concatenate([w_u, w_v], axis=0)                          # [128,TRI]
    # combined [xarg | w] per chunk, chunks back to back
    xw_a = np.empty((P, 2 * TRI), F32)
    for (r0, r1) in _a_chunks(6):
        o0, o1 = int(TRI_OFF[r0]), int(TRI_OFF[r1])
        xw_a[:, 2 * o0:o0 + o1] = xarg_a[:, o0:o1]
        xw_a[:, o0 + o1:2 * o1] = w_a[:, o0:o1]

    # ---- per-atom (B) layout, full problem then sliced per core ----
    gi_of = np.arange(T) // N
    li_of = np.arange(T) % N
    R_row = Rb[gi_of, li_of, :]                                       # [T,64]
    xargB = xargA[gi_of, li_of, :]
    invRB = invR[gi_of, li_of, :]
    tharg = (F32(1.2) - F32(0.2) * np.clip(R_row, R_IN, R_MAX)).astype(F32)

    t_i = types[:, None]                                              # [T,1]
    t_j = tb[gi_of]                                                   # [T,64]
    lnA = np.log(ref_A.astype(F32)).astype(F32)
    beta = ref_B.astype(F32)[t_i, t_j]
    delta = (beta * ref_mu.astype(F32)[t_i, t_j] + lnA[t_i, t_j]).astype(F32)
    earg = (delta - beta * R_row).astype(F32)
    iR2 = (invRB * invRB).astype(F32)
    iR6 = (iR2 * iR2 * iR2).astype(F32)
    cd = (ref_C.astype(F32)[t_i, t_j] * iR6
          + ref_D.astype(F32)[t_i, t_j] * iR6 * iR2).astype(F32)
    diag01 = (t_j * 0).astype(F32)
    diag01[np.arange(T), li_of] = 1.0
    earg[np.arange(T), li_of] = -100.0
    cd[np.arange(T), li_of] = 0.0

    dterm = (F32(0.5) / (sigma * SQRT_PI)).astype(F32)                # [T]
    ase = atomic_short_energy.astype(F32)[:, 0]                       # [T]

    # constant small tensors
    shift_sel = np.zeros((P, G), F32)
    shift_sel[np.arange(G) + G, np.arange(G)] = 1.0                   # k==m+64
    halves = np.zeros((P, 2), F32)
    halves[:G, 0] = 1.0
    halves[G:, 1] = 1.0

    def to_b_layout(arr_tc):  # [APC, 64] -> [128, 256], atom = s*128 + p
        return np.ascontiguousarray(
            arr_tc.reshape(S_SUB, P, N).transpose(1, 0, 2).reshape(P, BW))

    def to_b_small(vec):      # [APC] -> [128, 4]
        return np.ascontiguousarray(vec.reshape(S_SUB, P).T)

    in_maps = []
    for c in range(NCORES):
        rows = slice(c * APC, (c + 1) * APC)
        auxp = np.zeros((P, AUXW), F32)
        auxp[:, 0:BP] = np.concatenate(
            [to_b_layout(a[rows]) for a in (xargB, invRB, tharg, earg, cd, diag01)],
            axis=1)
        auxp[:, BP + 0:BP + 64] = shift_sel
        auxp[:, BP + 64:BP + 66] = halves
        auxp[:, BP + 66:BP + 70] = to_b_small(dterm[rows])
        auxp[:, BP + 70:BP + 74] = to_b_small(ase[rows])
        for s in range(S_SUB):
            pcol = np.arange(P)
            gsel = c * GPC + 2 * s + (pcol >= G).astype(np.int64)
            auxp[gsel, SEL0 + s * P + pcol] = 1.0
        in_maps.append(dict(
            xw_a=np.ascontiguousarray(xw_a),
            aux=np.ascontiguousarray(auxp),
        ))
    return in_maps


def _conforming(batch, edge_index, edge_length, node_attrs, kappa,
                short_energy, atomic_short_energy, node_feats, **kw):
    if batch.shape != (T,) or node_attrs.shape != (T, 4):
        return False
    if short_energy.shape != (G, 1) or node_feats.shape[0] != T:
        return False
    if not np.array_equal(np.asarray(batch, np.int64),
                          np.repeat(np.arange(G, dtype=np.int64), N)):
        return False
    e0 = np.asarray(edge_index[0], np.int64)
    e1 = np.asarray(edge_index[1], np.int64)
    if e0.min() < 0 or e0.max() >= T or e1.min() < 0 or e1.max() >= T:
        return False
    if not np.all(e0 // N == e1 // N):          # all edges within a graph
        return False
    el = np.asarray(edge_length, F32)
    if not np.all((el > 0) & (el <= R_MAX)):    # keeps F_cut branch-free
        return False
    return True


def kernel(**inputs):
    import concourse.bass_utils as bass_utils

    args = {k: np.asarray(v) for k, v in inputs.items()}
    if not _conforming(**args):
        return _fallback_numpy(**args)

    nc = _get_compiled()
    in_maps = _host_prep(**args)
    res = bass_utils.run_bass_kernel_spmd(nc, in_maps,
                                          core_ids=list(range(NCORES)))

    charges = np.ascontiguousarray(
        res.results[0]["q_out"].reshape(T)).astype(F32)
    pot = np.empty(T, F32)
    ne = np.empty(T, F32)
    E_el = np.empty(G, F32)
    E2b_raw = np.empty(G, F32)
    for c in range(NCORES):
        r = res.results[c]
        rows = slice(c * APC, (c + 1) * APC)
        pn = r["potne_out"]                          # [128, 8]
        pot[rows] = pn[:, 0:S_SUB].T.reshape(APC)    # atom = s*128 + p
        ne[rows] = pn[:, S_SUB:2 * S_SUB].T.reshape(APC)
        sums = r["sums_out"]                         # [2, 8]
        for s in range(S_SUB):
            for h in range(2):
                g = c * GPC + 2 * s + h
                E_el[g] = sums[h, s]
                E2b_raw[g] = sums[h, 4 + s]

    E_tot = (E_el + F32(0.5) * E2b_raw).reshape(G, 1) \
        + args["short_energy"].astype(F32)
    node_energy = ne[:, None]
    node_feats_out = np.concatenate(
        [args["node_feats"].astype(F32), charges[:, None], pot[:, None]],
        axis=1).astype(F32)
    return (E_tot.astype(F32), node_energy.astype(F32), charges,
            node_feats_out)


# revision 31
# speedup vs baseline: 1.3687x; 1.0183x over previous
"""Trainium2 Bass kernel for the MACE charge-equilibration module (nn_MACE_44435731645168).

Mathematical restructuring (exact, validated against the jax reference):
  * batch = repeat(arange(64), 64) and every edge connects atoms of the same
    graph, so the dense TxT distance matrix is EPS everywhere except inside the
    64 diagonal 64x64 blocks.  Off-block entries of `factor` and `F_cut` are the
    analytic constants c_off = erf(1/sqrt(2))/EPS and c1 = tanh(1)^3, and the
    off-block part of each dense matvec collapses to c_off*(S_tot - S_g) with
    S_g the per-graph charge sum.  All remaining work is per-graph 64x64 blocks.
  * The extended (N+1)x(N+1) solve [[L,1],[1^T,0]] [q;lam] = [-kappa;0] with L
    lower-triangular reduces to two triangular solves: u = L^-1 kappa,
    v = L^-1 1, q = -u + (sum(u)/sum(v)) v.  Both are done for all 64 graphs at
    once on-device: graphs x {u,v} occupy the 128 SBUF partitions and each
    forward-substitution step is a single fused multiply-reduce instruction.

Device work: erf/tanh/exp transcendentals for all pair blocks, the 64-step
batched triangular solve, the blocked matvecs/reductions, and the PE matmuls
that shuffle charges between layouts.  Host work: dtype/layout prep, scattering
the edge list into dense 64x64 blocks (last-write-wins, matching XLA scatter),
and expanding the tiny 4x4 type-pair parameter tables.

Each of the 8 cores runs an identical program; per-core inputs differ only in
which 8 graphs (512 atoms) the core computes outputs for.  The (tiny) solve is
replicated on every core, which removes all cross-core communication.
"""

import math

import numpy as np

try:
    from scipy.special import erf as _erf
except Exception:  # pragma: no cover
    _erf = np.vectorize(math.erf, otypes=[np.float64])

G = 64
N = 64
T = G * N
P = 128
NCORES = 8
GPC = G // NCORES        # graphs per core
APC = T // NCORES        # atoms per core
S_SUB = APC // P         # free-dim sub-blocks in the per-atom layout (4)
EPS = np.float32(0.5)
R_MAX = np.float32(6.0)
R_IN = np.float32(1.0)
SQRT_PI = np.float32(np.sqrt(np.pi))
C1 = np.float32(np.tanh(1.0) ** 3)
C_OFF = np.float32(_erf(1.0 / np.sqrt(2.0)) / 0.5)

F32 = np.float32

# bpack free-dim slices ([128, 256] each)
BW = S_SUB * N  # 256
_BP_NAMES = ["xargB", "invRB", "tharg", "earg", "cd", "diag01"]
BP = BW * len(_BP_NAMES)
# spack free-dim layout: shift_sel [0:64], halves [64:66], dterm [66:70], ase [70:74]
SP = 80
SEL0 = BP + SP          # selq offset inside aux pack
AUXW = SEL0 + S_SUB * P  # aux pack width

# packed lower-triangle A layout: row j occupies [tri(j), tri(j)+j+1) with
# slot 0 carrying b_tilde and slots 1..j carrying -L[j,k]/L[j,j], k<j
TRI_OFF = np.cumsum([0] + [j + 1 for j in range(N)])  # [65]; TRI_OFF[64]=2080
TRI = int(TRI_OFF[N])
_RIDX = np.repeat(np.arange(N), np.arange(1, N + 1))          # [TRI]
_CIDX = np.concatenate([np.arange(j + 1) for j in range(N)])  # [TRI]

_compiled = None


def _a_chunks(nch=None):
    """Packed-row chunks; first chunk small so the solve starts early."""
    bounds = [0, 16, 32, 48, N]
    return [(bounds[i], bounds[i + 1]) for i in range(len(bounds) - 1)]


def _build():
    """Build + compile the Bass program once. Returns (nc, meta)."""
    import concourse.bacc as bacc
    import concourse.tile as tile
    import concourse.mybir as mybir
    from concourse import bass

    dt = mybir.dt.float32
    Act = mybir.ActivationFunctionType
    Op = mybir.AluOpType
    Ax = mybir.AxisListType

    nc = bacc.Bacc("TRN2", target_bir_lowering=False, debug=False,
                   num_devices=NCORES)

    xw_a = nc.dram_tensor("xw_a", [P, 2 * TRI], dt, kind="ExternalInput").ap()
    aux = nc.dram_tensor("aux", [P, AUXW], dt, kind="ExternalInput").ap()

    q_out = nc.dram_tensor("q_out", [G, N], dt, kind="ExternalOutput").ap()
    potne_out = nc.dram_tensor("potne_out", [P, 2 * S_SUB], dt,
                               kind="ExternalOutput").ap()
    sums_out = nc.dram_tensor("sums_out", [2, 8], dt, kind="ExternalOutput").ap()

    from concourse.tile import add_dep_helper

    chunks = _a_chunks()    # packed A-stage row-group chunks

    def bslice(tile_ap, name):
        i = _BP_NAMES.index(name)
        return tile_ap[:, i * BW:(i + 1) * BW]

    with tile.TileContext(nc) as tc:
        with tc.tile_pool(name="main", bufs=1) as pool, \
             tc.tile_pool(name="chunks", bufs=3) as cpool, \
             tc.tile_pool(name="psum", bufs=2, space="PSUM") as pp:

            # ---------------- A stage: packed Lneg chunks ----------------
            # Packed rows: row j at [tri(j), tri(j)+j+1); slot 0 carries
            # b_tilde (via w = b_tilde/erf(1)), slots 1..j carry
            # -L[j,k]/L[j,j] for k<j.  One combined [xarg|w] DMA per chunk;
            # erf on Scalar, Lneg multiply on GpSimd so the Vector engine
            # stays free for the serial substitution chain.
            erf_chunks = []
            for ci, (r0, r1) in enumerate(chunks):
                o0, o1 = int(TRI_OFF[r0]), int(TRI_OFF[r1])
                cw = o1 - o0
                xw = cpool.tile([P, 2 * cw], dt, tag=f"xw{ci}")
                nc.sync.dma_start(out=xw[:], in_=xw_a[:, 2 * o0:2 * o1])
                ea = cpool.tile([P, cw], dt, tag=f"ea{ci}")
                ei = nc.scalar.activation(out=ea[:], in_=xw[:, 0:cw],
                                          func=Act.Erf)
                erf_chunks.append((r0, r1, o0, xw, ea, ei))

            # ---------------- aux pack DMA (after solve-critical ones) ----
            auxt = pool.tile([P, AUXW], dt)
            nc.sync.dma_start(out=auxt[:], in_=aux[:])
            bp = auxt[:, 0:BP]
            sp = auxt[:, BP:BP + SP]
            sel = auxt[0:G, SEL0:SEL0 + S_SUB * P]

            # ---------------- batched forward substitution ----------------
            # partitions = graph g (rows 0-63, RHS u) and g+64 (RHS v)
            # y layout: col 0 = constant 1, cols 1..64 = solution, col 65 = sum
            # The Lneg multiply for each chunk sits in the Vector stream just
            # before the first substitution step that consumes it.
            y = pool.tile([P, N + 2], dt)
            scratch = pool.tile([P, N], dt)
            nc.vector.memset(y[:, 0:1], 1.0)
            lneg = {}
            solve_steps = []
            for (r0, r1, o0, xw, ea, _ei) in erf_chunks:
                cw = int(TRI_OFF[r1]) - o0
                ln = pool.tile([P, cw], dt, tag=f"lneg{r0}")
                nc.vector.tensor_tensor(out=ln[:], in0=ea[:],
                                        in1=xw[:, cw:2 * cw], op=Op.mult)
                for j in range(r0, r1):
                    off = int(TRI_OFF[j]) - o0
                    si = nc.vector.scalar_tensor_tensor(
                        out=scratch[:, 0:j + 1],
                        in0=ln[:, off:off + j + 1],
                        scalar=1.0,
                        in1=y[:, 0:j + 1],
                        op0=Op.bypass,
                        op1=Op.mult,
                        accum_out=y[:, j + 1:j + 2],
                    )
                    solve_steps.append(si)
            # row sums -> y[:, 65]  (Sum u per graph / Sum v per graph)
            nc.vector.tensor_reduce(out=y[:, N + 1:N + 2], in_=y[:, 1:N + 1],
                                    axis=Ax.X, op=Op.add)

            # ---------------- q = -u + (Su/Sv) v ----------------
            vsh = pp.tile([G, N + 1], dt)
            nc.tensor.matmul(vsh[:], lhsT=sp[:, 0:64], rhs=y[:, 1:N + 2],
                             start=True, stop=True)
            rv = pool.tile([G, 1], dt)
            nc.vector.reciprocal(out=rv[:], in_=vsh[:, N:N + 1])
            r = pool.tile([G, 1], dt)
            nc.vector.tensor_tensor(out=r[:], in0=y[0:G, N + 1:N + 2], in1=rv[:],
                                    op=Op.mult)
            # qrhs: cols 0..63 = charges per graph, col 64 = S_tot - S_g
            qrhs = pool.tile([G, N + 1], dt)
            qa = qrhs[:, 0:N]
            nc.vector.scalar_tensor_tensor(out=qa, in0=vsh[:, 0:N],
                                           scalar=r[:], in1=y[0:G, 1:N + 1],
                                           op0=Op.mult, op1=Op.subtract)
            nc.sync.dma_start(out=q_out[:], in_=qa)

            # S_tot broadcast, then offw = S_tot - S_g into qrhs col 64
            sga = pool.tile([G, 1], dt)
            nc.vector.tensor_reduce(out=sga[:], in_=qa, axis=Ax.X, op=Op.add)
            ones64 = pool.tile([G, 1], dt)
            nc.vector.memset(ones64[:], 1.0)
            stp = pp.tile([1, 1], dt)
            nc.tensor.matmul(stp[:], lhsT=sga[:], rhs=ones64[:],
                             start=True, stop=True)
            sts = pool.tile([1, 1], dt)
            nc.vector.tensor_copy(out=sts[:], in_=stp[:])
            stb = pool.tile([G, 1], dt)
            nc.gpsimd.partition_broadcast(stb[:], sts[:])
            nc.vector.tensor_scalar(out=qrhs[:, N:N + 1], in0=sga[:],
                                    scalar1=stb[:], scalar2=-1.0,
                                    op0=Op.subtract, op1=Op.mult)

            # ---------------- charges + offw in per-atom layout ----------
            W1 = N + 1
            qbp = pp.tile([P, S_SUB * W1], dt)
            for s in range(S_SUB):
                nc.tensor.matmul(qbp[:, s * W1:(s + 1) * W1],
                                 lhsT=sel[:, s * P:(s + 1) * P], rhs=qrhs[:],
                                 start=True, stop=True)
            qbs = pool.tile([P, S_SUB * W1], dt)
            nc.vector.tensor_copy(out=qbs[:], in_=qbp[:])
            qb3 = qbs[:].rearrange("p (s w) -> p s w", w=W1)
            qB = qb3[:, :, 0:N]                   # [128, 4, 64] charges
            offw = qb3[:, :, N:N + 1]             # [128, 4, 1]  S_tot - S_g

            # ---------------- B stage (per-atom rows, this core's atoms) ----
            # force the activation-table order erf -> tanh -> exp (3 loads)
            eb = pool.tile([P, BW], dt)
            i_eb = nc.scalar.activation(out=eb[:], in_=bslice(bp, "xargB"),
                                        func=Act.Erf)
            add_dep_helper(erf_chunks[-1][5].ins, i_eb.ins, sync=False,
                           reason="B-erf after A-erfs")
            th = pool.tile([P, BW], dt)
            i_th = nc.scalar.activation(out=th[:], in_=bslice(bp, "tharg"),
                                        func=Act.Tanh)
            add_dep_helper(i_eb.ins, i_th.ins, sync=False,
                           reason="tanh after all erf")
            ex = pool.tile([P, BW], dt)
            i_ex = nc.scalar.activation(out=ex[:], in_=bslice(bp, "earg"),
                                        func=Act.Exp)
            add_dep_helper(i_th.ins, i_ex.ins, sync=False,
                           reason="exp after tanh")

            fac = pool.tile([P, BW], dt)
            nc.vector.tensor_tensor(out=fac[:], in0=eb[:],
                                    in1=bslice(bp, "invRB"), op=Op.mult)
            th2 = pool.tile([P, BW], dt)
            nc.vector.tensor_tensor(out=th2[:], in0=th[:], in1=th[:], op=Op.mult)
            fcut = pool.tile([P, BW], dt)
            nc.vector.tensor_tensor(out=fcut[:], in0=th2[:], in1=th[:], op=Op.mult)
            t2b = pool.tile([P, BW], dt)
            nc.vector.tensor_tensor(out=t2b[:], in0=ex[:], in1=bslice(bp, "cd"),
                                    op=Op.subtract)

            # products tile: [tmpv | tmpp | qdiag | e2b], one grouped reduce
            prods = pool.tile([P, 4 * BW], dt)
            tmpv = prods[:, 0:BW]
            tv3 = tmpv.rearrange("p (s l) -> p s l", l=N)
            nc.vector.tensor_tensor(out=tv3, in0=fac[:].rearrange(
                "p (s l) -> p s l", l=N), in1=qB, op=Op.mult)
            nc.vector.tensor_tensor(out=prods[:, BW:2 * BW], in0=tmpv,
                                    in1=fcut[:], op=Op.mult)
            nc.vector.tensor_tensor(
                out=prods[:, 2 * BW:3 * BW].rearrange("p (s l) -> p s l", l=N),
                in0=qB, in1=bslice(bp, "diag01").rearrange(
                    "p (s l) -> p s l", l=N), op=Op.mult)
            nc.vector.tensor_tensor(out=prods[:, 3 * BW:4 * BW], in0=t2b[:],
                                    in1=fcut[:], op=Op.mult)
            redx = pool.tile([P, 5 * S_SUB], dt)
            nc.vector.tensor_reduce(
                out=redx[:, 0:4 * S_SUB],
                in_=prods[:].rearrange("p (s l) -> p s l", l=N),
                axis=Ax.X, op=Op.add)
            vpre = redx[:, 0:S_SUB]
            pb = redx[:, S_SUB:2 * S_SUB]
            qown = redx[:, 2 * S_SUB:3 * S_SUB]
            e2brow = redx[:, 3 * S_SUB:4 * S_SUB]

            # atomic potentials: P + c1*c_off*(S_tot - S_g)
            potne = pool.tile([P, 2 * S_SUB], dt)
            pot = potne[:, 0:S_SUB]
            nc.vector.scalar_tensor_tensor(
                out=pot.unsqueeze(-1), in0=offw, scalar=float(C1 * C_OFF),
                in1=pb.unsqueeze(-1), op0=Op.mult, op1=Op.add)

            # V, atomic electrostatic energy
            v1 = pool.tile([P, S_SUB], dt)
            nc.vector.scalar_tensor_tensor(
                out=v1[:].unsqueeze(-1), in0=offw, scalar=float(C_OFF),
                in1=vpre.unsqueeze(-1), op0=Op.mult, op1=Op.add)
            dq = pool.tile([P, S_SUB], dt)
            nc.vector.tensor_tensor(out=dq[:], in0=qown, in1=sp[:, 66:70],
                                    op=Op.mult)
            vfin = pool.tile([P, S_SUB], dt)
            nc.vector.tensor_tensor(out=vfin[:], in0=v1[:], in1=dq[:], op=Op.add)
            aeel = redx[:, 4 * S_SUB:5 * S_SUB]
            nc.vector.tensor_tensor(out=aeel, in0=qown, in1=vfin[:], op=Op.mult)

            ne = pool.tile([P, S_SUB], dt)
            nc.vector.scalar_tensor_tensor(out=ne[:], in0=e2brow, scalar=0.5,
                                           in1=aeel, op0=Op.mult, op1=Op.add)
            ne2 = potne[:, S_SUB:2 * S_SUB]
            nc.vector.tensor_tensor(out=ne2, in0=ne[:], in1=sp[:, 70:74],
                                    op=Op.add)
            nc.sync.dma_start(out=potne_out[:], in_=potne[:])

            # per-graph sums of [e2brow | aeel]  (halves selector matmul)
            smp = pp.tile([2, 8], dt)
            nc.tensor.matmul(smp[:], lhsT=sp[:, 64:66],
                             rhs=redx[:, 3 * S_SUB:5 * S_SUB],
                             start=True, stop=True)
            sms = pool.tile([2, 8], dt)
            nc.vector.tensor_copy(out=sms[:], in_=smp[:])
            nc.sync.dma_start(out=sums_out[:], in_=sms[:])

    nc.compile()
    return nc


def _get_compiled():
    global _compiled
    if _compiled is None:
        _compiled = _build()
    return _compiled


def _f_cut_np(R):
    c1 = np.float32(np.tanh(1.0) ** 3)
    smooth = np.tanh((1.0 - (R - R_IN) / (R_MAX - R_IN)).astype(F32)).astype(F32) ** 3
    out = np.where((R > 0) & (R < R_IN), c1,
                   np.where((R >= R_IN) & (R <= R_MAX), smooth, F32(0.0)))
    return out.astype(F32)


def _fallback_numpy(batch, node_attrs, edge_index, edge_length, kappa,
                    node_feats, ref_eta, ref_log_sigma, ref_A, ref_B, ref_C,
                    ref_D, ref_mu, short_energy, atomic_short_energy):
    """Faithful numpy port of the reference for non-conforming inputs."""
    Tn = batch.shape[0]
    Gn = int(batch.max()) + 1 if Tn else 0
    Gn = max(Gn, short_energy.shape[0])
    Nn = Tn // Gn
    types = np.argmax(node_attrs, axis=1)
    ref_sigma = np.exp(ref_log_sigma).astype(F32)
    sigma = ref_sigma[types]
    eta = ref_eta[types]
    ref_gamma = np.sqrt(ref_sigma[:, None] ** 2 + ref_sigma[None, :] ** 2).astype(F32)
    same = batch[:, None] == batch[None, :]
    gamma = np.where(same, ref_gamma[types[:, None], types[None, :]], EPS).astype(F32)
    R = np.full((Tn, Tn), EPS, dtype=F32)
    R[edge_index[0], edge_index[1]] = edge_length[:, 0]
    Fc = _f_cut_np(R)
    factor = (_erf((R / (np.sqrt(F32(2.0)) * gamma)).astype(F32)).astype(F32) / R).astype(F32)
    A_ij = np.diag((eta + 1.0 / (sigma * SQRT_PI)).astype(F32)) + factor
    A_tril = np.tril(A_ij).astype(F32)
    A4 = A_tril.reshape(Gn, Nn, Gn, Nn)
    gi = np.arange(Gn)
    A_blocks = A4[gi, :, gi, :]
    A_ext = np.zeros((Gn, Nn + 1, Nn + 1), dtype=F32)
    A_ext[:, :Nn, :Nn] = A_blocks
    A_ext[:, Nn, :Nn] = 1.0
    A_ext[:, :Nn, Nn] = 1.0
    kappa_ext = np.concatenate([kappa.reshape(Gn, Nn),
                                np.zeros((Gn, 1), F32)], axis=1)
    sol = np.linalg.solve(A_ext.astype(np.float64),
                          -kappa_ext[..., None].astype(np.float64))[..., 0]
    charges = sol[:, :Nn].reshape(Tn).astype(F32)
    atomic_potentials = ((factor * Fc) @ charges).astype(F32)
    E_factor = np.diag((0.5 / (sigma * SQRT_PI)).astype(F32)) + factor
    V = (E_factor @ charges).astype(F32)
    atomic_E_el = (charges * V).astype(F32)
    E_el = np.zeros((Gn, 1), F32)
    np.add.at(E_el[:, 0], batch, atomic_E_el)
    pair = lambda ref: np.where(same, ref[types[:, None], types[None, :]], F32(0.0)).astype(F32)
    Am, Bm, Cm, Dm, mum = (pair(x) for x in (ref_A, ref_B, ref_C, ref_D, ref_mu))
    E2b_ij = ((Am * np.exp((Bm * (mum - R)).astype(F32)).astype(F32)
               - Cm / R ** 6 - Dm / R ** 8).astype(F32) * Fc)
    E2b_ij *= (1.0 - np.eye(Tn, dtype=F32))
    atomic_E2b = (E2b_ij.sum(axis=-1) * 0.5)[:, None].astype(F32)
    E_2b = np.zeros((Gn, 1), F32)
    np.add.at(E_2b[:, 0], batch, atomic_E2b[:, 0])
    E_tot = (E_el + E_2b + short_energy.astype(F32)).astype(F32)
    node_energy = (atomic_E_el[:, None] + atomic_E2b
                   + atomic_short_energy.astype(F32)).astype(F32)
    node_feats_out = np.concatenate(
        [node_feats.astype(F32), charges[:, None], atomic_potentials[:, None]],
        axis=1).astype(F32)
    return E_tot, node_energy, charges, node_feats_out


def _host_prep(batch, node_attrs, edge_index, edge_length, kappa, node_feats,
               ref_eta, ref_log_sigma, ref_A, ref_B, ref_C, ref_D, ref_mu,
               short_energy, atomic_short_energy):
    """Numpy-side layout/table prep. Returns per-core in_maps."""
    types = np.argmax(node_attrs, axis=1).astype(np.int64)            # [T]
    ref_sigma = np.exp(ref_log_sigma.astype(F32)).astype(F32)
    sigma = ref_sigma[types]
    eta = ref_eta.astype(F32)[types]
    ref_gamma = np.sqrt(ref_sigma[:, None] ** 2
                        + ref_sigma[None, :] ** 2).astype(F32)        # [4,4]

    # dense per-graph blocks of R (last-write-wins scatter, matches XLA)
    e0 = edge_index[0].astype(np.int64)
    e1 = edge_index[1].astype(np.int64)
    Rb = np.full((G, N, N), EPS, dtype=F32)
    Rb[e0 // N, e0 % N, e1 % N] = edge_length[:, 0].astype(F32)

    tb = types.reshape(G, N)                                          # [G,N]
    gam = ref_gamma[tb[:, :, None], tb[:, None, :]]                   # [G,N,N]
    inv_sqrt2g = (F32(1.0) / (np.sqrt(F32(2.0)) * gam)).astype(F32)
    xargA = (Rb * inv_sqrt2g).astype(F32)                             # [G,N,N]
    invR = (F32(1.0) / Rb).astype(F32)

    # diagonal of L and the row pre-scaling
    idx = np.arange(N)
    fdiag = (_erf(xargA[:, idx, idx]).astype(F32) * invR[:, idx, idx]).astype(F32)
    dvec = (eta + F32(1.0) / (sigma * SQRT_PI)).astype(F32).reshape(G, N)
    invd = (F32(1.0) / (dvec + fdiag)).astype(F32)                    # [G,N]
    wA = (-invd[:, :, None] * invR).astype(F32)                       # [G,N,N]

    # packed lower-triangle rows; slot 0 of row j carries b_tilde via
    # erf(1.0) * (b_tilde/erf(1.0)) = b_tilde
    erf1 = F32(_erf(np.float32(1.0)))
    bu = (invd * kappa.astype(F32).reshape(G, N) / erf1).astype(F32)
    bv = (invd / erf1).astype(F32)
    cid0 = np.maximum(_CIDX - 1, 0)
    half_x = np.where(_CIDX == 0, F32(1.0), xargA[:, _RIDX, cid0])    # [G,TRI]
    wgat = wA[:, _RIDX, cid0]
    w_u = np.where(_CIDX == 0, bu[:, _RIDX], wgat).astype(F32)
    w_v = np.where(_CIDX == 0, bv[:, _RIDX], wgat).astype(F32)
    xarg_a = np.concatenate([half_x, half_x], axis=0).astype(F32)     # [128,TRI]
    w_a = np.concatenate([w_u, w_v], axis=0)                          # [128,TRI]
    # combined [xarg | w] per chunk, chunks back to back
    xw_a = np.empty((P, 2 * TRI), F32)
    for (r0, r1) in _a_chunks(6):
        o0, o1 = int(TRI_OFF[r0]), int(TRI_OFF[r1])
        xw_a[:, 2 * o0:o0 + o1] = xarg_a[:, o0:o1]
        xw_a[:, o0 + o1:2 * o1] = w_a[:, o0:o1]

    # ---- per-atom (B) layout, full problem then sliced per core ----
    gi_of = np.arange(T) // N
    li_of = np.arange(T) % N
    R_row = Rb[gi_of, li_of, :]                                       # [T,64]
    xargB = xargA[gi_of, li_of, :]
    invRB = invR[gi_of, li_of, :]
    tharg = (F32(1.2) - F32(0.2) * np.clip(R_row, R_IN, R_MAX)).astype(F32)

    t_i = types[:, None]                                              # [T,1]
    t_j = tb[gi_of]                                                   # [T,64]
    lnA = np.log(ref_A.astype(F32)).astype(F32)
    beta = ref_B.astype(F32)[t_i, t_j]
    delta = (beta * ref_mu.astype(F32)[t_i, t_j] + lnA[t_i, t_j]).astype(F32)
    earg = (delta - beta * R_row).astype(F32)
    iR2 = (invRB * invRB).astype(F32)
    iR6 = (iR2 * iR2 * iR2).astype(F32)
    cd = (ref_C.astype(F32)[t_i, t_j] * iR6
          + ref_D.astype(F32)[t_i, t_j] * iR6 * iR2).astype(F32)
    diag01 = (t_j * 0).astype(F32)
    diag01[np.arange(T), li_of] = 1.0
    earg[np.arange(T), li_of] = -100.0
    cd[np.arange(T), li_of] = 0.0

    dterm = (F32(0.5) / (sigma * SQRT_PI)).astype(F32)                # [T]
    ase = atomic_short_energy.astype(F32)[:, 0]                       # [T]

    # constant small tensors
    shift_sel = np.zeros((P, G), F32)
    shift_sel[np.arange(G) + G, np.arange(G)] = 1.0                   # k==m+64
    halves = np.zeros((P, 2), F32)
    halves[:G, 0] = 1.0
    halves[G:, 1] = 1.0

    def to_b_layout(arr_tc):  # [APC, 64] -> [128, 256], atom = s*128 + p
        return np.ascontiguousarray(
            arr_tc.reshape(S_SUB, P, N).transpose(1, 0, 2).reshape(P, BW))

    def to_b_small(vec):      # [APC] -> [128, 4]
        return np.ascontiguousarray(vec.reshape(S_SUB, P).T)

    in_maps = []
    for c in range(NCORES):
        rows = slice(c * APC, (c + 1) * APC)
        auxp = np.zeros((P, AUXW), F32)
        auxp[:, 0:BP] = np.concatenate(
            [to_b_layout(a[rows]) for a in (xargB, invRB, tharg, earg, cd, diag01)],
            axis=1)
        auxp[:, BP + 0:BP + 64] = shift_sel
        auxp[:, BP + 64:BP + 66] = halves
        auxp[:, BP + 66:BP + 70] = to_b_small(dterm[rows])
        auxp[:, BP + 70:BP + 74] = to_b_small(ase[rows])
        for s in range(S_SUB):
            pcol = np.arange(P)
            gsel = c * GPC + 2 * s + (pcol >= G).astype(np.int64)
            auxp[gsel, SEL0 + s * P + pcol] = 1.0
        in_maps.append(dict(
            xw_a=np.ascontiguousarray(xw_a),
            aux=np.ascontiguousarray(auxp),
        ))
    return in_maps


def _conforming(batch, edge_index, edge_length, node_attrs, kappa,
                short_energy, atomic_short_energy, node_feats, **kw):
    if batch.shape != (T,) or node_attrs.shape != (T, 4):
        return False
    if short_energy.shape != (G, 1) or node_feats.shape[0] != T:
        return False
    if not np.array_equal(np.asarray(batch, np.int64),
                          np.repeat(np.arange(G, dtype=np.int64), N)):
        return False
    e0 = np.asarray(edge_index[0], np.int64)
    e1 = np.asarray(edge_index[1], np.int64)
    if e0.min() < 0 or e0.max() >= T or e1.min() < 0 or e1.max() >= T:
        return False
    if not np.all(e0 // N == e1 // N):          # all edges within a graph
        return False
    el = np.asarray(edge_length, F32)
    if not np.all((el > 0) & (el <= R_MAX)):    # keeps F_cut branch-free
        return False
    return True


def kernel(**inputs):
    import concourse.bass_utils as bass_utils

    args = {k: np.asarray(v) for k, v in inputs.items()}
    if not _conforming(**args):
        return _fallback_numpy(**args)

    nc = _get_compiled()
    in_maps = _host_prep(**args)
    res = bass_utils.run_bass_kernel_spmd(nc, in_maps,
                                          core_ids=list(range(NCORES)))

    charges = np.ascontiguousarray(
        res.results[0]["q_out"].reshape(T)).astype(F32)
    pot = np.empty(T, F32)
    ne = np.empty(T, F32)
    E_el = np.empty(G, F32)
    E2b_raw = np.empty(G, F32)
    for c in range(NCORES):
        r = res.results[c]
        rows = slice(c * APC, (c + 1) * APC)
        pn = r["potne_out"]                          # [128, 8]
        pot[rows] = pn[:, 0:S_SUB].T.reshape(APC)    # atom = s*128 + p
        ne[rows] = pn[:, S_SUB:2 * S_SUB].T.reshape(APC)
        sums = r["sums_out"]                         # [2, 8]
        for s in range(S_SUB):
            for h in range(2):
                g = c * GPC + 2 * s + h
                E2b_raw[g] = sums[h, s]
                E_el[g] = sums[h, 4 + s]

    E_tot = (E_el + F32(0.5) * E2b_raw).reshape(G, 1) \
        + args["short_energy"].astype(F32)
    node_energy = ne[:, None]
    node_feats_out = np.concatenate(
        [args["node_feats"].astype(F32), charges[:, None], pot[:, None]],
        axis=1).astype(F32)
    return (E_tot.astype(F32), node_energy.astype(F32), charges,
            node_feats_out)
